# revision 16
# baseline (speedup 1.0000x reference)
"""GATv2 x2 + BN + classifier GNN on 8 trn2 NeuronCores.

Nodes are dst-sharded 6250/core; each core owns the edges pointing at its
nodes, grouped into 49 blocks of 128 dst nodes and padded to one uniform
tile count so all 8 cores run a single SPMD graph.  Per-edge xl/xr rows
are fetched with dma_gather spread round-robin over 4 SWDGE queues.  The
segment softmax + aggregation run through a per-tile one-hot Mt matrix on
the TensorEngine with the payload as the stationary operand, so the
aggregate lands CHANNEL-major ([C, dst]); softmax division, BN and ReLU
then use per-partition scalars, and the classifier is a plain matmul on
the channel-major hidden state.  Self-loops are applied densely at node
level.  Layer-2 tables are exchanged with an AllGather.
"""

import sys

sys.path.insert(0, "/opt/trn_rl_repo")

import numpy as np
import ml_dtypes

import bass_rust
import concourse.bass as bass
import concourse.bacc as bacc
import concourse.tile as tile
import concourse.mybir as mybir
from concourse.bass_utils import run_bass_kernel_spmd

f32 = mybir.dt.float32
bf16 = mybir.dt.bfloat16
i16 = mybir.dt.int16
AF = mybir.ActivationFunctionType
ALU = mybir.AluOpType
AX = mybir.AxisListType

N, E, IN, HID, HEADS, OUT = 50000, 800000, 128, 32, 4, 2
NEG = 0.2
EPS = 1e-5
NC = 8
NPC = N // NC                 # 6250
NBLK = (NPC + 127) // 128     # 49 (last block has 106 nodes)
HC = HEADS * HID              # 128
HALF = 25000                  # src-table split point (int16 idx range)

_CACHE = {}


# ---------------------------------------------------------------- host prep
def _wrap_idx(idx):
    """int [n] -> int16 [128, n//16]; token i at [i%16, i//16], replicated
    to all 8 Q7 core groups."""
    n = idx.shape[0]
    w = idx.astype(np.int16).reshape(n // 16, 16).T
    return np.ascontiguousarray(np.tile(w, (8, 1)))


def host_prep(edge_index, edge_attr):
    src = np.asarray(edge_index[0]).astype(np.int64)
    dst = np.asarray(edge_index[1]).astype(np.int64)
    ea = np.asarray(edge_attr[:, 0], np.float32)
    core_of = dst // NPC

    per_core = []
    t_lo = t_hi = 1
    for c in range(NC):
        m = core_of == c
        s_c, d_c, a_c = src[m], dst[m] - c * NPC, ea[m]
        blk = d_c // 128
        groups = []
        for b in range(NBLK):
            mb = blk == b
            mlo = mb & (s_c < HALF)
            mhi = mb & (s_c >= HALF)
            groups.append((s_c[mlo], d_c[mlo], a_c[mlo],
                           s_c[mhi], d_c[mhi], a_c[mhi]))
            t_lo = max(t_lo, (int(mlo.sum()) + 127) // 128)
            t_hi = max(t_hi, (int(mhi.sum()) + 127) // 128)
        per_core.append(groups)

    TB = t_lo + t_hi
    maps = []
    for c in range(NC):
        idx_lo = np.zeros((NBLK, t_lo * 128), np.int64)
        idx_hi = np.zeros((NBLK, t_hi * 128), np.int64)
        idx_dst = np.zeros((NBLK, TB * 128), np.int64)
        drel = np.full((NBLK, TB * 128), -1.0, np.float32)
        eatt = np.zeros((NBLK, TB * 128), np.float32)
        for b, (sl, dl, al, sh, dh, ah) in enumerate(per_core[c]):
            nl, nh = len(sl), len(sh)
            o = t_lo * 128
            idx_lo[b, :nl] = sl
            idx_hi[b, :nh] = sh - HALF
            idx_dst[b, :nl] = dl
            idx_dst[b, o:o + nh] = dh
            drel[b, :nl] = dl - b * 128
            drel[b, o:o + nh] = dh - b * 128
            eatt[b, :nl] = al
            eatt[b, o:o + nh] = ah
        d4 = drel.reshape(NBLK, TB, 128)          # [b, t, e]
        oneh = d4[:, :, :, None] == np.arange(128)[None, None, None, :]
        mt = oneh.transpose(2, 0, 1, 3)           # [e, b, t, d]
        mtt = oneh.transpose(3, 0, 1, 2)          # [d, b, t, e]
        maps.append(dict(
            idx_lo=_wrap_idx(idx_lo.reshape(-1)),
            idx_hi=_wrap_idx(idx_hi.reshape(-1)),
            idx_dst=_wrap_idx(idx_dst.reshape(-1)),
            mt_w=np.ascontiguousarray(
                mt.reshape(128, -1).astype(ml_dtypes.bfloat16)),
            mtt_w=np.ascontiguousarray(
                mtt.reshape(128, -1).astype(ml_dtypes.bfloat16)),
            eattr_w=np.ascontiguousarray(eatt.reshape(-1, 128).T),
        ))
    return maps, t_lo, t_hi


# ---------------------------------------------------------------- device
def build(t_lo, t_hi):
    TB = t_lo + t_hi
    TT = NBLK * TB
    nc = bacc.Bacc("TRN2", target_bir_lowering=False, debug=False,
                   num_devices=NC, num_swdge_queues=4)

    def din(name, shape, dt=f32):
        return nc.declare_dram_parameter(name, list(shape), dt, isOutput=False)

    xT = nc.declare_dram_parameter("xT", [128, N], bf16, isOutput=False)
    xTo = nc.declare_dram_parameter("xTo", [128, NPC], bf16, isOutput=False)
    ea_full = din("ea_full", [128, E // 128])
    idx_lo_d = din("idx_lo", [128, NBLK * t_lo * 8], i16)
    idx_hi_d = din("idx_hi", [128, NBLK * t_hi * 8], i16)
    mt_d = nc.declare_dram_parameter("mt_w", [128, TT * 128], bf16,
                                     isOutput=False)
    mtt_d = nc.declare_dram_parameter("mtt_w", [128, TT * 128], bf16,
                                      isOutput=False)
    eattr_d = din("eattr_w", [128, TT])

    w = {k: din(k, sh) for k, sh in [
        ("c1_Wl", (IN, HC)), ("c1_Wr", (IN, HC)),
        ("c2_Wl", (HC, HID)), ("c2_Wr", (HC, HID)),
        ("clf_W", (HID, OUT)), ("clf_b", (OUT, 1)),
        ("c1_We", (1, HC)), ("c1_att", (1, HC)),
        ("c2_We", (1, HID)), ("c2_att", (1, HID)),
        # host-computed columns / masks
        ("we1_col", (HC, 1)), ("blbr1_col", (HC, 1)),
        ("pre1_col", (HC, 1)), ("bnsc1_col", (HC, 1)), ("pbb1_col", (HC, 1)),
        ("we2_col", (HID, 1)), ("blbr2_col", (HID, 1)),
        ("pre2_col", (HID, 1)), ("bnsc2_col", (HID, 1)), ("pbb2_col", (HID, 1)),
        ("headmask1", (HEADS, HC)), ("hmsc1", (HEADS, HC)),
        ("blbr1_row", (1, HC)),
    ]}
    am1 = nc.declare_dram_parameter("att_mask1", [HC, HEADS], bf16, isOutput=False)
    am2 = nc.declare_dram_parameter("att_mask2", [HID, HID + 1], bf16,
                                    isOutput=False)
    hm2_d = nc.declare_dram_parameter("headmask2", [HID + 1, HID], f32,
                                      isOutput=False)
    hmsc2_d = nc.declare_dram_parameter("hmsc2", [HID + 1, HID], f32,
                                        isOutput=False)
    out_T = nc.declare_dram_parameter("out_T", [OUT, NPC], f32, isOutput=True)

    xl_tbl = nc.dram_tensor("xl_tbl", [N, 128], bf16)
    t2_stripe = nc.dram_tensor("t2_stripe", [NPC, 128], bf16)
    t2_full = nc.dram_tensor("t2_full", [N, 128], bf16, addr_space="Shared")

    with tile.TileContext(nc) as tc:
        stack = []

        def pool(name, bufs, space="SBUF"):
            p = tc.tile_pool(name=name, bufs=bufs, space=space)
            stack.append(p)
            return p.__enter__()

        cst = pool("cst", 1)
        pp = pool("pp", 6, "PSUM")       # f32 psum (matmul accum)
        pb = pool("pb", 2, "PSUM")       # bf16 psum (transposes)

        def psum():
            return pp.tile([128, 512], f32, tag="ps", name="ps")

        def psum_bf():
            return pb.tile([128, 1024], bf16, tag="psb", name="psb")

        # ---- constants ----------------------------------------------------
        ones1f = cst.tile([1, 128], f32, tag="ones1f")
        nc.gpsimd.memset(ones1f[:], 1.0)

        rows = {}
        for k, wd in [("c1_We", HC), ("c1_att", HC),
                      ("c2_We", HID), ("c2_att", HID)]:
            t = cst.tile([1, wd], f32, tag=f"row_{k}")
            nc.sync.dma_start(t[:], w[k][:])
            rows[k] = t

        cols = {}
        for k, wd in [("we1_col", HC), ("blbr1_col", HC), ("pre1_col", HC),
                      ("bnsc1_col", HC), ("pbb1_col", HC),
                      ("we2_col", HID), ("blbr2_col", HID),
                      ("pre2_col", HID), ("bnsc2_col", HID),
                      ("pbb2_col", HID), ("clf_b", OUT)]:
            t = cst.tile([wd, 1], f32, tag=f"col_{k}")
            nc.sync.dma_start(t[:], (w[k] if k != "clf_b" else w["clf_b"])[:])
            cols[k] = t

        hm1 = cst.tile([HEADS, HC], f32, tag="hm1")
        nc.sync.dma_start(hm1[:], w["headmask1"][:])
        hmsc1 = cst.tile([HEADS, HC], f32, tag="hmsc1")
        nc.sync.dma_start(hmsc1[:], w["hmsc1"][:])
        am1_sb = cst.tile([HC, HEADS], bf16, tag="am1")
        nc.sync.dma_start(am1_sb[:], am1[:])
        am2_aug = cst.tile([HID, HID + 1], bf16, tag="am2")
        nc.sync.dma_start(am2_aug[:], am2[:])
        hm2_aug = cst.tile([HID + 1, HID], f32, tag="hm2")
        nc.sync.dma_start(hm2_aug[:], hm2_d[:])
        hmsc2 = cst.tile([HID + 1, HID], f32, tag="hmsc2")
        nc.sync.dma_start(hmsc2[:], hmsc2_d[:])
        rec2_t = cst.tile([HID + 1, 128], f32, tag="rec2")
        nc.gpsimd.memset(rec2_t[:], 1.0)
        blbr1_row = cst.tile([1, HC], f32, tag="blbr1_row")
        nc.sync.dma_start(blbr1_row[:], w["blbr1_row"][:])

        def rep(tag, row, wd, dt=bf16):
            ps = psum()
            nc.tensor.matmul(out=ps[:, :wd], lhsT=ones1f[:],
                             rhs=row[:, :wd], start=True, stop=True)
            t = cst.tile([128, wd], dt, tag=f"rep_{tag}")
            nc.vector.tensor_copy(t[:], ps[:, :wd])
            return t

        We1_rep = rep("We1", rows["c1_We"], HC)
        We2_rep = rep("We2", rows["c2_We"], HID)
        att1_rep = rep("att1", rows["c1_att"], HC)
        att2_rep = rep("att2", rows["c2_att"], HID)
        blbr1_rep = rep("blbr1", blbr1_row, HC, f32)

        def big(tag, base, wd):
            t = cst.tile([128, TB * wd], bf16, tag=f"big_{tag}")
            for i in range(TB):
                nc.sync.dma_start(t[:, i * wd:(i + 1) * wd], base[:])
            return t

        att1_big = big("att1", att1_rep, HC)
        att2_big = big("att2", att2_rep, HID)
        We1_big = big("We1", We1_rep, HC)
        We2_big = big("We2", We2_rep, HID)

        iota_i = cst.tile([128, 128], i16, tag="iota_i")
        nc.gpsimd.iota(iota_i[:], pattern=[[1, 128]], base=0,
                       channel_multiplier=0)
        iota_rep = cst.tile([128, 128], bf16, tag="iota_rep")
        nc.vector.tensor_copy(iota_rep[:], iota_i[:])
        icol_i = cst.tile([128, 1], i16, tag="icol_i")
        nc.gpsimd.iota(icol_i[:], pattern=[[1, 1]], base=0,
                       channel_multiplier=1)
        icol_f = cst.tile([128, 1], f32, tag="icol_f")
        nc.vector.tensor_copy(icol_f[:], icol_i[:])
        ident = cst.tile([128, 128], bf16, tag="ident")
        nc.vector.tensor_scalar(ident[:], iota_rep[:], icol_f[:], None,
                                op0=ALU.is_equal)

        # mean of edge_attr
        with tc.tile_pool(name="eaf", bufs=1) as eaf:
            ea_sb = eaf.tile([128, E // 128], f32, tag="ea")
            nc.sync.dma_start(ea_sb[:], ea_full[:])
            ea_part = cst.tile([128, 1], f32, tag="ea_part")
            nc.vector.tensor_reduce(ea_part[:], ea_sb[:], axis=AX.X,
                                    op=ALU.add)
        ea_sum = cst.tile([128, 1], f32, tag="ea_sum")
        nc.gpsimd.partition_all_reduce(ea_sum[:], ea_part[:], 128,
                                       bass_rust.ReduceOp.add)
        mean_sc = cst.tile([128, 1], f32, tag="mean_sc")
        nc.vector.tensor_scalar(mean_sc[:], ea_sum[:], 1.0 / E, None,
                                op0=ALU.mult)
        wm1_col = cst.tile([HC, 1], f32, tag="wm1_col")
        nc.vector.tensor_tensor(wm1_col[:], cols["we1_col"][:], mean_sc[0:HC],
                                op=ALU.mult)
        wm2_col = cst.tile([HID, 1], f32, tag="wm2_col")
        nc.vector.tensor_tensor(wm2_col[:], cols["we2_col"][:], mean_sc[0:HID],
                                op=ALU.mult)

        wcat1 = cst.tile([128, 2 * HC], bf16, tag="wcat1")
        nc.gpsimd.dma_start(wcat1[:, 0:HC], w["c1_Wl"][:])
        nc.gpsimd.dma_start(wcat1[:, HC:2 * HC], w["c1_Wr"][:])
        wcat2 = cst.tile([128, 2 * HID], bf16, tag="wcat2")
        nc.gpsimd.dma_start(wcat2[:, 0:HID], w["c2_Wl"][:])
        nc.gpsimd.dma_start(wcat2[:, HID:2 * HID], w["c2_Wr"][:])
        clfW = cst.tile([HID, OUT], bf16, tag="clfW")
        nc.gpsimd.dma_start(clfW[:], w["clf_W"][:])

        meta = pool("meta", 1)
        ilo_sb = meta.tile([128, NBLK * t_lo * 8], i16, tag="ilo")
        nc.sync.dma_start(ilo_sb[:], idx_lo_d[:])
        ihi_sb = meta.tile([128, NBLK * t_hi * 8], i16, tag="ihi")
        nc.sync.dma_start(ihi_sb[:], idx_hi_d[:])
        eattr_bf = meta.tile([128, TT], bf16, tag="eattr_bf")
        with tc.tile_pool(name="mf32", bufs=1) as mf32:
            eattr_sb = mf32.tile([128, TT], f32, tag="eattr")
            nc.sync.dma_start(eattr_sb[:], eattr_d[:])
            nc.vector.tensor_copy(eattr_bf[:], eattr_sb[:])

        # ---- phase A: xl table (node-major rows for gather) ---------------
        CH = 1024
        keep = pool("keep", 1)
        # channel-major own-node transforms (for self-loops / L2 tables)
        xl1T = keep.tile([128, NBLK * 128], bf16, tag="xl1T")
        xr1T = keep.tile([128, NBLK * 128], bf16, tag="xr1T")
        xr1_own = keep.tile([128, NBLK * 128], bf16, tag="xr1_own")
        h1T = keep.tile([128, NBLK * 128], bf16, tag="h1T")
        for t_ in (xl1T, xr1T, h1T):
            nc.gpsimd.memset(t_[:], 0.0)
        # L2 aliases (L1 contents are dead by the time these are written)
        xl2T = xl1T
        xr2T = xr1T
        h2T = h1T
        st2_own = keep.tile([128, NBLK * 2 * HID], bf16, tag="st2_own")

        xa = pool("xa", 2)
        for ci in range((N + CH - 1) // CH):
            n0 = ci * CH
            n1 = min(n0 + CH, N)
            nfull = (n1 - n0) // 128
            xTs = xa.tile([128, CH], bf16, tag="xTs")
            nc.sync.dma_start(xTs[:, :n1 - n0], xT[:, n0:n1])
            stg = xa.tile([128, CH // 128, 128], bf16, tag="stage")
            for t in range((n1 - n0 + 127) // 128):
                m = min(128, n1 - n0 - t * 128)
                ps = psum()
                nc.tensor.matmul(out=ps[:m, 0:HC],
                                 lhsT=xTs[:, t * 128:t * 128 + m],
                                 rhs=wcat1[:, 0:HC], start=True, stop=True)
                if t % 2 == 0:
                    nc.scalar.activation(stg[:m, t, :], ps[:m, 0:HC],
                                         AF.Identity)
                else:
                    nc.vector.tensor_copy(stg[:m, t, :], ps[:m, 0:HC])
            if nfull:
                nc.sync.dma_start(
                    xl_tbl[n0:n0 + nfull * 128, :].rearrange(
                        "(t p) c -> p t c", p=128),
                    stg[:, 0:nfull, :])
            if (n1 - n0) % 128:
                m = (n1 - n0) % 128
                nc.sync.dma_start(xl_tbl[n0 + nfull * 128:n1, :],
                                  stg[:m, nfull, :])

        ob = pool("ob", 2)
        xTos = ob.tile([128, NPC], bf16, tag="xTos", bufs=1)
        nc.sync.dma_start(xTos[:], xTo[:])
        for b in range(NBLK):
            m = min(128, NPC - b * 128)
            # node-major xr (for the one-hot xr matmul)
            ps = psum()
            nc.tensor.matmul(out=ps[:m, 0:HC],
                             lhsT=xTos[:, b * 128:b * 128 + m],
                             rhs=wcat1[:, HC:2 * HC], start=True, stop=True)
            nc.vector.tensor_tensor(xr1_own[:m, b * 128:(b + 1) * 128],
                                    ps[:m, 0:HC], blbr1_rep[:m, :],
                                    op=ALU.add)
            # channel-major own transforms
            ps2 = psum()
            nc.tensor.matmul(out=ps2[:, 0:m], lhsT=wcat1[:, 0:HC],
                             rhs=xTos[:, b * 128:b * 128 + m],
                             start=True, stop=True)
            nc.vector.tensor_copy(xl1T[:, b * 128:b * 128 + m],
                                  ps2[:, 0:m])
            ps3 = psum()
            nc.tensor.matmul(out=ps3[:, 0:m], lhsT=wcat1[:, HC:2 * HC],
                             rhs=xTos[:, b * 128:b * 128 + m],
                             start=True, stop=True)
            nc.vector.tensor_scalar(xr1T[:, b * 128:b * 128 + m],
                                    ps3[:, 0:m], cols["blbr1_col"][:], None,
                                    op0=ALU.add)

        # ---- edge phase ----------------------------------------------------
        ep = pool("ep", 2)
        eb = pool("eb", 3)
        mq = pool("mq", 3)
        l2 = pool("l2", 2)
        fin = pool("fin", 2)
        QN = [0]

        def l2_table_build(b):
            ps = psum()
            nc.tensor.matmul(out=ps[0:HID, 0:128], lhsT=wcat2[:, 0:HID],
                             rhs=h1T[:, b * 128:(b + 1) * 128],
                             start=True, stop=True)
            nc.vector.tensor_copy(xl2T[0:HID, b * 128:(b + 1) * 128],
                                  ps[0:HID, 0:128])
            ps2 = psum()
            nc.tensor.matmul(out=ps2[0:HID, 0:128], lhsT=wcat2[:, HID:2 * HID],
                             rhs=h1T[:, b * 128:(b + 1) * 128],
                             start=True, stop=True)
            nc.vector.tensor_scalar(xr2T[0:HID, b * 128:(b + 1) * 128],
                                    ps2[0:HID, 0:128],
                                    cols["blbr2_col"][:], None, op0=ALU.add)
            tps = psum_bf()
            nc.tensor.transpose(tps[:, 0:HID],
                                xl2T[0:HID, b * 128:(b + 1) * 128],
                                ident[0:HID, 0:HID])
            nc.tensor.transpose(tps[:, HID:2 * HID],
                                xr2T[0:HID, b * 128:(b + 1) * 128],
                                ident[0:HID, 0:HID])
            nc.vector.tensor_copy(st2_own[:, b * 2 * HID:(b + 1) * 2 * HID],
                                  tps[:, 0:2 * HID])
            m = min(128, NPC - b * 128)
            nc.sync.dma_start(
                t2_stripe[b * 128:b * 128 + m, 0:2 * HID],
                st2_own[0:m, b * 2 * HID:(b + 1) * 2 * HID])

        def clf_block(b):
            m = min(128, NPC - b * 128)
            ps = psum()
            nc.tensor.matmul(out=ps[0:OUT, 0:m], lhsT=clfW[:],
                             rhs=h2T[0:HID, b * 128:b * 128 + m],
                             start=True, stop=True)
            ot = fin.tile([OUT, 128], f32, tag="ot")
            nc.scalar.activation(ot[:, 0:m], ps[0:OUT, 0:m], AF.Identity,
                                 bias=cols["clf_b"][:])
            nc.sync.dma_start(out_T[:, b * 128:b * 128 + m], ot[:, 0:m])

        def edge_layer(C, nh, tbl_lo, tbl_hi, We_big_t, att_big_t, xr_rhs,
                       xlT_own, xrT_own, wm_col, pbb_col, hmsc_t,
                       am_sb, hm_t, h_out, fold, rec1, post_block):
            CA = C + 1 if fold else C
            for b in range(NBLK):
                xlg = ep.tile([128, TB, 128], bf16, tag="xlg")

                def gchunk(dst_t, toff, ntile, tbl, isrc, coff):
                    done = 0
                    while done < ntile:
                        k = min(8, ntile - done)
                        nc.gpsimd.dma_gather(
                            dst_t[:, toff + done:toff + done + k, :], tbl,
                            isrc[:, coff + done * 8:coff + (done + k) * 8],
                            k * 128, k * 128, 128, elem_step=128,
                            queue_num=QN[0] % 4)
                        QN[0] += 1
                        done += k

                gchunk(xlg, 0, t_lo, tbl_lo, ilo_sb, b * t_lo * 8)
                gchunk(xlg, t_lo, t_hi, tbl_hi, ihi_sb, b * t_hi * 8)
                Mt = mq.tile([128, TB, 128], bf16, tag="mt")
                nc.sync.dma_start(
                    Mt[:], mt_d[:, b * TB * 128:(b + 1) * TB * 128]
                    .rearrange("p (t d) -> p t d", d=128))
                MtT = mq.tile([128, TB, 128], bf16, tag="mtt")
                nc.sync.dma_start(
                    MtT[:], mtt_d[:, b * TB * 128:(b + 1) * TB * 128]
                    .rearrange("p (t e) -> p t e", e=128))
                xl_u = xlg[:, :, 0:C]

                # xr[dst] per edge via one-hot matmul from node-major xr
                xr_e = ep.tile([128, TB, C], bf16, tag="xre")
                for t in range(TB):
                    xps = psum()
                    nc.tensor.matmul(out=xps[:, 0:C], lhsT=MtT[:, t, :],
                                     rhs=xr_rhs(b), start=True, stop=True)
                    nc.scalar.activation(xr_e[:, t, :], xps[:, 0:C],
                                         AF.Identity)

                ea_bc = eattr_bf[:, b * TB:(b + 1) * TB].to_broadcast(
                    [128, TB, C])
                el = eb.tile([128, TB, C], bf16, tag="ebuf")
                nc.vector.tensor_tensor(
                    el[:], We_big_t[:, 0:TB * C].rearrange(
                        "p (t c) -> p t c", c=C), ea_bc, op=ALU.mult)
                s1 = eb.tile([128, TB, C], bf16, tag="ebuf")
                nc.vector.tensor_tensor(s1[:], el[:], xl_u, op=ALU.add)
                s2 = eb.tile([128, TB, C], bf16, tag="ebuf")
                nc.vector.tensor_tensor(s2[:], s1[:], xr_e[:], op=ALU.add)
                lr = eb.tile([128, TB, C], bf16, tag="ebuf")
                nc.vector.scalar_tensor_tensor(
                    lr[:], s2[:], NEG, s2[:], op0=ALU.mult, op1=ALU.max)
                z = eb.tile([128, TB, C], bf16, tag="ebuf")
                nc.vector.tensor_tensor(
                    z[:], lr[:],
                    att_big_t[:, 0:TB * C].rearrange("p (t c) -> p t c", c=C),
                    op=ALU.mult)
                logit = ep.tile([128, TB * nh], f32, tag="logit")
                nc.vector.tensor_reduce(
                    logit[:].rearrange("p (t h) -> p t h", h=nh),
                    z[:].rearrange("p t (h c) -> p t h c", h=nh),
                    axis=AX.X, op=ALU.add)
                pay = eb.tile([128, TB, CA], bf16, tag="ebuf")
                if fold:
                    elog_ap = pay[:, :, C:C + 1].rearrange(
                        "p t one -> p (t one)")
                else:
                    elog_t = ep.tile([128, TB * nh], bf16, tag="elog")
                    elog_ap = elog_t[:]
                nc.scalar.activation(elog_ap, logit[:], AF.Exp)
                nc.vector.tensor_tensor(
                    pay[:, :, 0:C].rearrange("p t (h c) -> p t h c", h=nh),
                    xl_u.rearrange("p t (h c) -> p t h c", h=nh),
                    elog_ap.rearrange("p (t h) -> p t h", h=nh)
                        .to_broadcast([128, TB, nh, C // nh]),
                    op=ALU.mult)

                accT = psum()
                if fold:
                    for t in range(TB):
                        nc.tensor.matmul(out=accT[0:CA, 0:128],
                                         lhsT=pay[:, t, :], rhs=Mt[:, t, :],
                                         start=(t == 0), stop=(t == TB - 1))
                else:
                    accD = psum()
                    for t in range(TB):
                        nc.tensor.matmul(out=accT[0:C, 0:128],
                                         lhsT=pay[:, t, :], rhs=Mt[:, t, :],
                                         start=(t == 0), stop=(t == TB - 1))
                        nc.tensor.matmul(out=accD[0:nh, 0:128],
                                         lhsT=elog_t[:, t * nh:(t + 1) * nh],
                                         rhs=Mt[:, t, :],
                                         start=(t == 0), stop=(t == TB - 1))

                # dense self-loop (channel-major)
                xlb = xlT_own[0:C, b * 128:(b + 1) * 128]
                xrb = xrT_own[0:C, b * 128:(b + 1) * 128]
                s_l = ep.tile([C, 128], bf16, tag="s_l")
                nc.vector.scalar_tensor_tensor(
                    s_l[:], xlb, wm_col[:], xrb, op0=ALU.add, op1=ALU.add)
                lr_l = ep.tile([C, 128], bf16, tag="lr_l")
                nc.vector.scalar_tensor_tensor(
                    lr_l[:], s_l[:], NEG, s_l[:], op0=ALU.mult, op1=ALU.max)
                lgl = psum()
                nc.tensor.matmul(out=lgl[0:CA if fold else nh, 0:128],
                                 lhsT=am_sb[0:C, 0:CA if fold else nh],
                                 rhs=lr_l[:], start=True, stop=True)
                if fold:
                    Q = ep.tile([CA, 128], f32, tag="Q")
                    nc.scalar.activation(Q[:], lgl[0:CA, 0:128], AF.Exp)
                    elbc = psum()
                    nc.tensor.matmul(out=elbc[0:C, 0:128],
                                     lhsT=hm_t[0:CA, 0:C], rhs=Q[:],
                                     start=True, stop=True)
                    nc.vector.tensor_tensor(Q[0:C, :], xlb,
                                            elbc[0:C, 0:128], op=ALU.mult)
                    nim = ep.tile([CA, 128], f32, tag="nim")
                    nc.vector.tensor_tensor(nim[:], accT[0:CA, 0:128], Q[:],
                                            op=ALU.add)
                    nc.vector.reciprocal(rec1[C:CA, :], nim[C:CA, :])
                    rbc = psum()
                    nc.tensor.matmul(out=rbc[0:C, 0:128],
                                     lhsT=hmsc_t[0:CA, 0:C], rhs=rec1[:],
                                     start=True, stop=True)
                    o1 = ep.tile([C, 128], f32, tag="o1")
                    nc.vector.tensor_tensor(o1[:], nim[0:C, :],
                                            rbc[0:C, 0:128], op=ALU.mult)
                else:
                    elog_l = ep.tile([nh, 128], f32, tag="elog_l")
                    nc.scalar.activation(elog_l[:], lgl[0:nh, 0:128], AF.Exp)
                    pay_l = ep.tile([C, 128], f32, tag="pay_l")
                    elbc = psum()
                    nc.tensor.matmul(out=elbc[0:C, 0:128],
                                     lhsT=hm_t[0:nh, 0:C],
                                     rhs=elog_l[:], start=True, stop=True)
                    nc.vector.tensor_tensor(pay_l[:], xlb, elbc[0:C, 0:128],
                                            op=ALU.mult)
                    nim = ep.tile([C, 128], f32, tag="nim")
                    nc.vector.tensor_tensor(nim[:], accT[0:C, 0:128],
                                            pay_l[:], op=ALU.add)
                    den = ep.tile([nh, 128], f32, tag="den")
                    nc.vector.tensor_tensor(den[:], accD[0:nh, 0:128],
                                            elog_l[:], op=ALU.add)
                    rec = ep.tile([nh, 128], f32, tag="rec")
                    nc.vector.reciprocal(rec[:], den[:])
                    rbc = psum()
                    nc.tensor.matmul(out=rbc[0:C, 0:128],
                                     lhsT=hmsc_t[0:nh, 0:C],
                                     rhs=rec[:], start=True, stop=True)
                    o1 = ep.tile([C, 128], f32, tag="o1")
                    nc.vector.tensor_tensor(o1[:], nim[:], rbc[0:C, 0:128],
                                            op=ALU.mult)
                nc.scalar.activation(h_out[0:C, b * 128:(b + 1) * 128],
                                     o1[:], AF.Relu, bias=pbb_col[:])
                if post_block is not None:
                    post_block(b)

        edge_layer(HC, HEADS, xl_tbl[0:HALF, :], xl_tbl[HALF:N, :],
                   We1_big, att1_big,
                   lambda b: xr1_own[:, b * 128:(b + 1) * 128],
                   xl1T, xr1T, wm1_col, cols["pbb1_col"], hmsc1,
                   am1_sb, hm1, h1T, False, None, l2_table_build)

        nc.gpsimd.collective_compute(
            "AllGather", ALU.bypass,
            replica_groups=[list(range(NC))],
            ins=[t2_stripe.ap().opt()],
            outs=[t2_full.ap().opt()])

        edge_layer(HID, 1, t2_full[0:HALF, :], t2_full[HALF:N, :],
                   We2_big, att2_big,
                   lambda b: st2_own[:, b * 2 * HID + HID:(b + 1) * 2 * HID],
                   xl2T, xr2T, wm2_col, cols["pbb2_col"], hmsc2,
                   am2_aug, hm2_aug, h2T, True, rec2_t, clf_block)

        for p in reversed(stack):
            p.__exit__(None, None, None)

    nc.compile()
    return nc


# ---------------------------------------------------------------- entry
def make_in_maps(inputs, maps):
    f = lambda k: np.asarray(inputs[k], np.float32)
    x = f("x")
    edge_attr = f("edge_attr")
    xT = np.ascontiguousarray(x.T.astype(ml_dtypes.bfloat16))
    ea_full = np.ascontiguousarray(edge_attr[:, 0].reshape(E // 128, 128).T)

    # host-computed columns
    bl1, br1, bias1 = f("c1_bl"), f("c1_br"), f("c1_bias")
    g1, b1, m1, v1 = f("bn1_gamma"), f("bn1_beta"), f("bn1_mean"), f("bn1_var")
    bl2, br2, bias2 = f("c2_bl"), f("c2_br"), f("c2_bias")
    g2, b2, m2, v2 = f("bn2_gamma"), f("bn2_beta"), f("bn2_mean"), f("bn2_var")
    col = lambda a: np.ascontiguousarray(a.reshape(-1, 1).astype(np.float32))

    hm1 = np.zeros((HEADS, HC), np.float32)
    for h in range(HEADS):
        hm1[h, h * HID:(h + 1) * HID] = 1.0
    am1 = np.zeros((HC, HEADS), np.float32)
    att1 = f("c1_att")
    for h in range(HEADS):
        am1[h * HID:(h + 1) * HID, h] = att1[h]
    am2 = np.zeros((HID, HID + 1), np.float32)
    am2[:, HID] = f("c2_att")[0]
    hm2 = np.zeros((HID + 1, HID), np.float32)
    hm2[HID, :] = 1.0

    common = dict(
        xT=xT, ea_full=ea_full,
        we1_col=col(f("c1_We")[0]), blbr1_col=col(bl1 + br1),
        pre1_col=col(bl1 + bias1 - m1),
        bnsc1_col=col(g1 / np.sqrt(v1 + EPS)),
        pbb1_col=col((bl1 + bias1 - m1) * (g1 / np.sqrt(v1 + EPS)) + b1),
        we2_col=col(f("c2_We")[0]), blbr2_col=col(bl2 + br2),
        pre2_col=col(bl2 + bias2 - m2),
        bnsc2_col=col(g2 / np.sqrt(v2 + EPS)),
        pbb2_col=col((bl2 + bias2 - m2) * (g2 / np.sqrt(v2 + EPS)) + b2),
        headmask1=np.ascontiguousarray(hm1),
        hmsc1=np.ascontiguousarray(hm1 * (g1 / np.sqrt(v1 + EPS))[None, :]),
        att_mask1=np.ascontiguousarray(am1.astype(ml_dtypes.bfloat16)),
        att_mask2=np.ascontiguousarray(am2.astype(ml_dtypes.bfloat16)),
        headmask2=np.ascontiguousarray(hm2),
        hmsc2=np.ascontiguousarray(hm2 * (g2 / np.sqrt(v2 + EPS))[None, :]),
        blbr1_row=np.ascontiguousarray((bl1 + br1).reshape(1, -1)),
        clf_b=np.ascontiguousarray(f("clf_b").reshape(-1, 1)),
    )
    for k in ["c1_Wl", "c1_Wr", "c2_Wl", "c2_Wr", "clf_W"]:
        common[k] = np.ascontiguousarray(f(k))
    for k in ["c1_We", "c1_att", "c2_We", "c2_att"]:
        common[k] = np.ascontiguousarray(f(k).reshape(1, -1))

    in_maps = []
    for c in range(NC):
        m = dict(maps[c])
        m.update(common)
        m["xTo"] = np.ascontiguousarray(xT[:, c * NPC:(c + 1) * NPC])
        in_maps.append(m)
    return in_maps


def kernel(**inputs):
    edge_index = np.asarray(inputs["edge_index"])
    edge_attr = np.asarray(inputs["edge_attr"], np.float32)

    maps, t_lo, t_hi = host_prep(edge_index, edge_attr)
    key = (t_lo, t_hi)
    if key not in _CACHE:
        _CACHE[key] = build(t_lo, t_hi)
    nc = _CACHE[key]

    in_maps = make_in_maps(inputs, maps)
    res = run_bass_kernel_spmd(nc, in_maps, core_ids=list(range(NC)))
    global LAST_RESULT
    LAST_RESULT = res
    out = np.concatenate(
        [np.ascontiguousarray(np.asarray(r["out_T"]).T)
         for r in res.results], axis=0)
    return out.astype(np.float32)


# revision 17
# speedup vs baseline: 1.0624x; 1.0624x over previous
"""GATv2 x2 + BN + classifier GNN on 8 trn2 NeuronCores.

Nodes are dst-sharded 6250/core; each core owns the edges pointing at its
nodes, grouped into 49 blocks of 128 dst nodes and padded to one uniform
tile count so all 8 cores run a single SPMD graph.  Per-edge xl/xr rows
are fetched with dma_gather spread round-robin over 4 SWDGE queues.  The
segment softmax + aggregation run through a per-tile one-hot Mt matrix on
the TensorEngine with the payload as the stationary operand, so the
aggregate lands CHANNEL-major ([C, dst]); softmax division, BN and ReLU
then use per-partition scalars, and the classifier is a plain matmul on
the channel-major hidden state.  Self-loops are applied densely at node
level.  Layer-2 tables are exchanged with an AllGather.
"""

import sys

sys.path.insert(0, "/opt/trn_rl_repo")

import numpy as np
import ml_dtypes

import bass_rust
import concourse.bass as bass
import concourse.bacc as bacc
import concourse.tile as tile
import concourse.mybir as mybir
from concourse.bass_utils import run_bass_kernel_spmd

f32 = mybir.dt.float32
bf16 = mybir.dt.bfloat16
i16 = mybir.dt.int16
AF = mybir.ActivationFunctionType
ALU = mybir.AluOpType
AX = mybir.AxisListType

N, E, IN, HID, HEADS, OUT = 50000, 800000, 128, 32, 4, 2
NEG = 0.2
EPS = 1e-5
NC = 8
NPC = N // NC                 # 6250
NBLK = (NPC + 127) // 128     # 49 (last block has 106 nodes)
HC = HEADS * HID              # 128
HALF = 25000                  # src-table split point (int16 idx range)

_CACHE = {}


# ---------------------------------------------------------------- host prep
def _wrap_idx(idx):
    """int [n] -> int16 [128, n//16]; token i at [i%16, i//16], replicated
    to all 8 Q7 core groups."""
    n = idx.shape[0]
    w = idx.astype(np.int16).reshape(n // 16, 16).T
    return np.ascontiguousarray(np.tile(w, (8, 1)))


def host_prep(edge_index, edge_attr):
    src = np.asarray(edge_index[0]).astype(np.int64)
    dst = np.asarray(edge_index[1]).astype(np.int64)
    ea = np.asarray(edge_attr[:, 0], np.float32)
    core_of = dst // NPC

    per_core = []
    t_lo = t_hi = 1
    for c in range(NC):
        m = core_of == c
        s_c, d_c, a_c = src[m], dst[m] - c * NPC, ea[m]
        blk = d_c // 128
        groups = []
        for b in range(NBLK):
            mb = blk == b
            mlo = mb & (s_c < HALF)
            mhi = mb & (s_c >= HALF)
            groups.append((s_c[mlo], d_c[mlo], a_c[mlo],
                           s_c[mhi], d_c[mhi], a_c[mhi]))
            t_lo = max(t_lo, (int(mlo.sum()) + 127) // 128)
            t_hi = max(t_hi, (int(mhi.sum()) + 127) // 128)
        per_core.append(groups)

    TB = t_lo + t_hi
    maps = []
    for c in range(NC):
        idx_lo = np.zeros((NBLK, t_lo * 128), np.int64)
        idx_hi = np.zeros((NBLK, t_hi * 128), np.int64)
        idx_dst = np.zeros((NBLK, TB * 128), np.int64)
        drel = np.full((NBLK, TB * 128), -1.0, np.float32)
        eatt = np.zeros((NBLK, TB * 128), np.float32)
        for b, (sl, dl, al, sh, dh, ah) in enumerate(per_core[c]):
            nl, nh = len(sl), len(sh)
            o = t_lo * 128
            idx_lo[b, :nl] = sl
            idx_hi[b, :nh] = sh - HALF
            idx_dst[b, :nl] = dl
            idx_dst[b, o:o + nh] = dh
            drel[b, :nl] = dl - b * 128
            drel[b, o:o + nh] = dh - b * 128
            eatt[b, :nl] = al
            eatt[b, o:o + nh] = ah
        d4 = drel.reshape(NBLK, TB, 128)          # [b, t, e]
        oneh = d4[:, :, :, None] == np.arange(128)[None, None, None, :]
        mt = oneh.transpose(2, 0, 1, 3)           # [e, b, t, d]
        mtt = oneh.transpose(3, 0, 1, 2)          # [d, b, t, e]
        maps.append(dict(
            idx_lo=_wrap_idx(idx_lo.reshape(-1)),
            idx_hi=_wrap_idx(idx_hi.reshape(-1)),
            idx_dst=_wrap_idx(idx_dst.reshape(-1)),
            mt_w=np.ascontiguousarray(
                mt.reshape(128, -1).astype(ml_dtypes.bfloat16)),
            mtt_w=np.ascontiguousarray(
                mtt.reshape(128, -1).astype(ml_dtypes.bfloat16)),
            eattr_w=np.ascontiguousarray(eatt.reshape(-1, 128).T),
        ))
    return maps, t_lo, t_hi


# ---------------------------------------------------------------- device
def build(t_lo, t_hi):
    TB = t_lo + t_hi
    TT = NBLK * TB
    nc = bacc.Bacc("TRN2", target_bir_lowering=False, debug=False,
                   num_devices=NC, num_swdge_queues=4)

    def din(name, shape, dt=f32):
        return nc.declare_dram_parameter(name, list(shape), dt, isOutput=False)

    xT = nc.declare_dram_parameter("xT", [128, N], bf16, isOutput=False)
    xTo = nc.declare_dram_parameter("xTo", [128, NPC], bf16, isOutput=False)
    ea_full = din("ea_full", [128, E // 128])
    idx_lo_d = din("idx_lo", [128, NBLK * t_lo * 8], i16)
    idx_hi_d = din("idx_hi", [128, NBLK * t_hi * 8], i16)
    mt_d = nc.declare_dram_parameter("mt_w", [128, TT * 128], bf16,
                                     isOutput=False)
    mtt_d = nc.declare_dram_parameter("mtt_w", [128, TT * 128], bf16,
                                      isOutput=False)
    eattr_d = din("eattr_w", [128, TT])

    w = {k: din(k, sh) for k, sh in [
        ("c1_Wl", (IN, HC)), ("c1_Wr", (IN, HC)),
        ("c2_Wl", (HC, HID)), ("c2_Wr", (HC, HID)),
        ("clf_W", (HID, OUT)), ("clf_b", (OUT, 1)),
        ("c1_We", (1, HC)), ("c1_att", (1, HC)),
        ("c2_We", (1, HID)), ("c2_att", (1, HID)),
        # host-computed columns / masks
        ("we1_col", (HC, 1)), ("blbr1_col", (HC, 1)),
        ("pre1_col", (HC, 1)), ("bnsc1_col", (HC, 1)), ("pbb1_col", (HC, 1)),
        ("we2_col", (HID, 1)), ("blbr2_col", (HID, 1)),
        ("pre2_col", (HID, 1)), ("bnsc2_col", (HID, 1)), ("pbb2_col", (HID, 1)),
        ("headmask1", (HEADS, HC)), ("hmsc1", (HEADS, HC)),
        ("blbr1_row", (1, HC)),
    ]}
    am1 = nc.declare_dram_parameter("att_mask1", [HC, HEADS], bf16, isOutput=False)
    am2 = nc.declare_dram_parameter("att_mask2", [HID, HID + 1], bf16,
                                    isOutput=False)
    hm2_d = nc.declare_dram_parameter("headmask2", [HID + 1, HID], f32,
                                      isOutput=False)
    hmsc2_d = nc.declare_dram_parameter("hmsc2", [HID + 1, HID], f32,
                                        isOutput=False)
    out_T = nc.declare_dram_parameter("out_T", [OUT, NPC], f32, isOutput=True)

    xl_tbl = nc.dram_tensor("xl_tbl", [N, 128], bf16)
    t2_stripe = nc.dram_tensor("t2_stripe", [NPC, 128], bf16)
    t2_full = nc.dram_tensor("t2_full", [N, 128], bf16, addr_space="Shared")

    with tile.TileContext(nc) as tc:
        stack = []

        def pool(name, bufs, space="SBUF"):
            p = tc.tile_pool(name=name, bufs=bufs, space=space)
            stack.append(p)
            return p.__enter__()

        cst = pool("cst", 1)
        pp = pool("pp", 6, "PSUM")       # f32 psum (matmul accum)
        pb = pool("pb", 2, "PSUM")       # bf16 psum (transposes)

        def psum():
            return pp.tile([128, 512], f32, tag="ps", name="ps")

        def psum_bf():
            return pb.tile([128, 1024], bf16, tag="psb", name="psb")

        # ---- constants ----------------------------------------------------
        ones1f = cst.tile([1, 128], f32, tag="ones1f")
        nc.gpsimd.memset(ones1f[:], 1.0)

        rows = {}
        for k, wd in [("c1_We", HC), ("c1_att", HC),
                      ("c2_We", HID), ("c2_att", HID)]:
            t = cst.tile([1, wd], f32, tag=f"row_{k}")
            nc.sync.dma_start(t[:], w[k][:])
            rows[k] = t

        cols = {}
        for k, wd in [("we1_col", HC), ("blbr1_col", HC), ("pre1_col", HC),
                      ("bnsc1_col", HC), ("pbb1_col", HC),
                      ("we2_col", HID), ("blbr2_col", HID),
                      ("pre2_col", HID), ("bnsc2_col", HID),
                      ("pbb2_col", HID), ("clf_b", OUT)]:
            t = cst.tile([wd, 1], f32, tag=f"col_{k}")
            nc.sync.dma_start(t[:], (w[k] if k != "clf_b" else w["clf_b"])[:])
            cols[k] = t

        hm1 = cst.tile([HEADS, HC], f32, tag="hm1")
        nc.sync.dma_start(hm1[:], w["headmask1"][:])
        hmsc1 = cst.tile([HEADS, HC], f32, tag="hmsc1")
        nc.sync.dma_start(hmsc1[:], w["hmsc1"][:])
        am1_sb = cst.tile([HC, HEADS], bf16, tag="am1")
        nc.sync.dma_start(am1_sb[:], am1[:])
        am2_aug = cst.tile([HID, HID + 1], bf16, tag="am2")
        nc.sync.dma_start(am2_aug[:], am2[:])
        hm2_aug = cst.tile([HID + 1, HID], f32, tag="hm2")
        nc.sync.dma_start(hm2_aug[:], hm2_d[:])
        hmsc2 = cst.tile([HID + 1, HID], f32, tag="hmsc2")
        nc.sync.dma_start(hmsc2[:], hmsc2_d[:])
        rec2_t = cst.tile([HID + 1, 128], f32, tag="rec2")
        nc.gpsimd.memset(rec2_t[:], 1.0)
        blbr1_row = cst.tile([1, HC], f32, tag="blbr1_row")
        nc.sync.dma_start(blbr1_row[:], w["blbr1_row"][:])

        def rep(tag, row, wd, dt=bf16):
            ps = psum()
            nc.tensor.matmul(out=ps[:, :wd], lhsT=ones1f[:],
                             rhs=row[:, :wd], start=True, stop=True)
            t = cst.tile([128, wd], dt, tag=f"rep_{tag}")
            nc.vector.tensor_copy(t[:], ps[:, :wd])
            return t

        We1_rep = rep("We1", rows["c1_We"], HC)
        We2_rep = rep("We2", rows["c2_We"], HID)
        att1_rep = rep("att1", rows["c1_att"], HC)
        att2_rep = rep("att2", rows["c2_att"], HID)
        blbr1_rep = rep("blbr1", blbr1_row, HC, f32)

        def big(tag, base, wd):
            t = cst.tile([128, TB * wd], bf16, tag=f"big_{tag}")
            for i in range(TB):
                nc.sync.dma_start(t[:, i * wd:(i + 1) * wd], base[:])
            return t

        att1_big = big("att1", att1_rep, HC)
        att2_big = big("att2", att2_rep, HID)
        We1_big = big("We1", We1_rep, HC)
        We2_big = big("We2", We2_rep, HID)

        iota_i = cst.tile([128, 128], i16, tag="iota_i")
        nc.gpsimd.iota(iota_i[:], pattern=[[1, 128]], base=0,
                       channel_multiplier=0)
        iota_rep = cst.tile([128, 128], bf16, tag="iota_rep")
        nc.vector.tensor_copy(iota_rep[:], iota_i[:])
        icol_i = cst.tile([128, 1], i16, tag="icol_i")
        nc.gpsimd.iota(icol_i[:], pattern=[[1, 1]], base=0,
                       channel_multiplier=1)
        icol_f = cst.tile([128, 1], f32, tag="icol_f")
        nc.vector.tensor_copy(icol_f[:], icol_i[:])
        ident = cst.tile([128, 128], bf16, tag="ident")
        nc.vector.tensor_scalar(ident[:], iota_rep[:], icol_f[:], None,
                                op0=ALU.is_equal)

        # mean of edge_attr
        with tc.tile_pool(name="eaf", bufs=1) as eaf:
            ea_sb = eaf.tile([128, E // 128], f32, tag="ea")
            nc.sync.dma_start(ea_sb[:], ea_full[:])
            ea_part = cst.tile([128, 1], f32, tag="ea_part")
            nc.vector.tensor_reduce(ea_part[:], ea_sb[:], axis=AX.X,
                                    op=ALU.add)
        ea_sum = cst.tile([128, 1], f32, tag="ea_sum")
        nc.gpsimd.partition_all_reduce(ea_sum[:], ea_part[:], 128,
                                       bass_rust.ReduceOp.add)
        mean_sc = cst.tile([128, 1], f32, tag="mean_sc")
        nc.vector.tensor_scalar(mean_sc[:], ea_sum[:], 1.0 / E, None,
                                op0=ALU.mult)
        wm1_col = cst.tile([HC, 1], f32, tag="wm1_col")
        nc.vector.tensor_tensor(wm1_col[:], cols["we1_col"][:], mean_sc[0:HC],
                                op=ALU.mult)
        wm2_col = cst.tile([HID, 1], f32, tag="wm2_col")
        nc.vector.tensor_tensor(wm2_col[:], cols["we2_col"][:], mean_sc[0:HID],
                                op=ALU.mult)

        wcat1 = cst.tile([128, 2 * HC], bf16, tag="wcat1")
        nc.gpsimd.dma_start(wcat1[:, 0:HC], w["c1_Wl"][:])
        nc.gpsimd.dma_start(wcat1[:, HC:2 * HC], w["c1_Wr"][:])
        wcat2 = cst.tile([128, 2 * HID], bf16, tag="wcat2")
        nc.gpsimd.dma_start(wcat2[:, 0:HID], w["c2_Wl"][:])
        nc.gpsimd.dma_start(wcat2[:, HID:2 * HID], w["c2_Wr"][:])
        clfW = cst.tile([HID, OUT], bf16, tag="clfW")
        nc.gpsimd.dma_start(clfW[:], w["clf_W"][:])

        meta = pool("meta", 1)
        ilo_sb = meta.tile([128, NBLK * t_lo * 8], i16, tag="ilo")
        nc.sync.dma_start(ilo_sb[:], idx_lo_d[:])
        ihi_sb = meta.tile([128, NBLK * t_hi * 8], i16, tag="ihi")
        nc.sync.dma_start(ihi_sb[:], idx_hi_d[:])
        eattr_bf = meta.tile([128, TT], bf16, tag="eattr_bf")
        with tc.tile_pool(name="mf32", bufs=1) as mf32:
            eattr_sb = mf32.tile([128, TT], f32, tag="eattr")
            nc.sync.dma_start(eattr_sb[:], eattr_d[:])
            nc.vector.tensor_copy(eattr_bf[:], eattr_sb[:])

        # ---- phase A: xl table (node-major rows for gather) ---------------
        CH = 1024
        keep = pool("keep", 1)
        # channel-major own-node transforms (for self-loops / L2 tables)
        xl1T = keep.tile([128, NBLK * 128], bf16, tag="xl1T")
        xr1T = keep.tile([128, NBLK * 128], bf16, tag="xr1T")
        xr1_own = keep.tile([128, NBLK * 128], bf16, tag="xr1_own")
        h1T = keep.tile([128, NBLK * 128], bf16, tag="h1T")
        for t_ in (xl1T, xr1T, h1T):
            nc.gpsimd.memset(t_[:], 0.0)
        # L2 aliases (L1 contents are dead by the time these are written)
        xl2T = xl1T
        xr2T = xr1T
        h2T = h1T
        st2_own = keep.tile([128, NBLK * 2 * HID], bf16, tag="st2_own")

        xa = pool("xa", 2)
        for ci in range((N + CH - 1) // CH):
            n0 = ci * CH
            n1 = min(n0 + CH, N)
            nfull = (n1 - n0) // 128
            xTs = xa.tile([128, CH], bf16, tag="xTs")
            nc.sync.dma_start(xTs[:, :n1 - n0], xT[:, n0:n1])
            stg = xa.tile([128, CH // 128, 128], bf16, tag="stage")
            for t in range((n1 - n0 + 127) // 128):
                m = min(128, n1 - n0 - t * 128)
                ps = psum()
                nc.tensor.matmul(out=ps[:m, 0:HC],
                                 lhsT=xTs[:, t * 128:t * 128 + m],
                                 rhs=wcat1[:, 0:HC], start=True, stop=True)
                if t % 2 == 0:
                    nc.scalar.activation(stg[:m, t, :], ps[:m, 0:HC],
                                         AF.Identity)
                else:
                    nc.vector.tensor_copy(stg[:m, t, :], ps[:m, 0:HC])
            if nfull:
                nc.sync.dma_start(
                    xl_tbl[n0:n0 + nfull * 128, :].rearrange(
                        "(t p) c -> p t c", p=128),
                    stg[:, 0:nfull, :])
            if (n1 - n0) % 128:
                m = (n1 - n0) % 128
                nc.sync.dma_start(xl_tbl[n0 + nfull * 128:n1, :],
                                  stg[:m, nfull, :])

        ob = pool("ob", 2)
        xTos = ob.tile([128, NPC], bf16, tag="xTos", bufs=1)
        nc.sync.dma_start(xTos[:], xTo[:])
        for b in range(NBLK):
            m = min(128, NPC - b * 128)
            # node-major xr (for the one-hot xr matmul)
            ps = psum()
            nc.tensor.matmul(out=ps[:m, 0:HC],
                             lhsT=xTos[:, b * 128:b * 128 + m],
                             rhs=wcat1[:, HC:2 * HC], start=True, stop=True)
            nc.vector.tensor_tensor(xr1_own[:m, b * 128:(b + 1) * 128],
                                    ps[:m, 0:HC], blbr1_rep[:m, :],
                                    op=ALU.add)
            # channel-major own transforms
            ps2 = psum()
            nc.tensor.matmul(out=ps2[:, 0:m], lhsT=wcat1[:, 0:HC],
                             rhs=xTos[:, b * 128:b * 128 + m],
                             start=True, stop=True)
            nc.vector.tensor_copy(xl1T[:, b * 128:b * 128 + m],
                                  ps2[:, 0:m])
            ps3 = psum()
            nc.tensor.matmul(out=ps3[:, 0:m], lhsT=wcat1[:, HC:2 * HC],
                             rhs=xTos[:, b * 128:b * 128 + m],
                             start=True, stop=True)
            nc.vector.tensor_scalar(xr1T[:, b * 128:b * 128 + m],
                                    ps3[:, 0:m], cols["blbr1_col"][:], None,
                                    op0=ALU.add)

        # ---- edge phase ----------------------------------------------------
        ep = pool("ep", 3)
        eb = pool("eb", 4)
        mq = pool("mq", 3)
        l2 = pool("l2", 2)
        fin = pool("fin", 2)
        QN = [0]

        def l2_table_build(b):
            ps = psum()
            nc.tensor.matmul(out=ps[0:HID, 0:128], lhsT=wcat2[:, 0:HID],
                             rhs=h1T[:, b * 128:(b + 1) * 128],
                             start=True, stop=True)
            nc.vector.tensor_copy(xl2T[0:HID, b * 128:(b + 1) * 128],
                                  ps[0:HID, 0:128])
            ps2 = psum()
            nc.tensor.matmul(out=ps2[0:HID, 0:128], lhsT=wcat2[:, HID:2 * HID],
                             rhs=h1T[:, b * 128:(b + 1) * 128],
                             start=True, stop=True)
            nc.vector.tensor_scalar(xr2T[0:HID, b * 128:(b + 1) * 128],
                                    ps2[0:HID, 0:128],
                                    cols["blbr2_col"][:], None, op0=ALU.add)
            tps = psum_bf()
            nc.tensor.transpose(tps[:, 0:HID],
                                xl2T[0:HID, b * 128:(b + 1) * 128],
                                ident[0:HID, 0:HID])
            nc.tensor.transpose(tps[:, HID:2 * HID],
                                xr2T[0:HID, b * 128:(b + 1) * 128],
                                ident[0:HID, 0:HID])
            nc.vector.tensor_copy(st2_own[:, b * 2 * HID:(b + 1) * 2 * HID],
                                  tps[:, 0:2 * HID])
            m = min(128, NPC - b * 128)
            nc.sync.dma_start(
                t2_stripe[b * 128:b * 128 + m, 0:2 * HID],
                st2_own[0:m, b * 2 * HID:(b + 1) * 2 * HID])

        def clf_block(b):
            m = min(128, NPC - b * 128)
            ps = psum()
            nc.tensor.matmul(out=ps[0:OUT, 0:m], lhsT=clfW[:],
                             rhs=h2T[0:HID, b * 128:b * 128 + m],
                             start=True, stop=True)
            ot = fin.tile([OUT, 128], f32, tag="ot")
            nc.scalar.activation(ot[:, 0:m], ps[0:OUT, 0:m], AF.Identity,
                                 bias=cols["clf_b"][:])
            nc.sync.dma_start(out_T[:, b * 128:b * 128 + m], ot[:, 0:m])

        def edge_layer(C, nh, tbl_lo, tbl_hi, We_big_t, att_big_t, xr_rhs,
                       xlT_own, xrT_own, wm_col, pbb_col, hmsc_t,
                       am_sb, hm_t, h_out, fold, rec1, post_block):
            CA = C + 1 if fold else C
            for b in range(NBLK):
                xlg = ep.tile([128, TB, 128], bf16, tag="xlg")

                def gchunk(dst_t, toff, ntile, tbl, isrc, coff):
                    done = 0
                    while done < ntile:
                        k = min(8, ntile - done)
                        nc.gpsimd.dma_gather(
                            dst_t[:, toff + done:toff + done + k, :], tbl,
                            isrc[:, coff + done * 8:coff + (done + k) * 8],
                            k * 128, k * 128, 128, elem_step=128,
                            queue_num=QN[0] % 4)
                        QN[0] += 1
                        done += k

                gchunk(xlg, 0, t_lo, tbl_lo, ilo_sb, b * t_lo * 8)
                gchunk(xlg, t_lo, t_hi, tbl_hi, ihi_sb, b * t_hi * 8)
                Mt = mq.tile([128, TB, 128], bf16, tag="mt")
                nc.sync.dma_start(
                    Mt[:], mt_d[:, b * TB * 128:(b + 1) * TB * 128]
                    .rearrange("p (t d) -> p t d", d=128))
                MtT = mq.tile([128, TB, 128], bf16, tag="mtt")
                nc.sync.dma_start(
                    MtT[:], mtt_d[:, b * TB * 128:(b + 1) * TB * 128]
                    .rearrange("p (t e) -> p t e", e=128))
                xl_u = xlg[:, :, 0:C]

                # xr[dst] per edge via one-hot matmul from node-major xr
                xr_e = ep.tile([128, TB, C], bf16, tag="xre")
                for t in range(TB):
                    xps = psum()
                    nc.tensor.matmul(out=xps[:, 0:C], lhsT=MtT[:, t, :],
                                     rhs=xr_rhs(b), start=True, stop=True)
                    if t % 2 == 0:
                        nc.scalar.activation(xr_e[:, t, :], xps[:, 0:C],
                                             AF.Identity)
                    else:
                        nc.vector.tensor_copy(xr_e[:, t, :], xps[:, 0:C])

                ea_bc = eattr_bf[:, b * TB:(b + 1) * TB].to_broadcast(
                    [128, TB, C])
                ch = eb.tile([128, TB, C], bf16, tag="ebuf")
                nc.vector.tensor_tensor(
                    ch[:], We_big_t[:, 0:TB * C].rearrange(
                        "p (t c) -> p t c", c=C), ea_bc, op=ALU.mult)
                nc.vector.tensor_tensor(ch[:], ch[:], xl_u, op=ALU.add)
                nc.vector.tensor_tensor(ch[:], ch[:], xr_e[:], op=ALU.add)
                nc.vector.scalar_tensor_tensor(
                    ch[:], ch[:], NEG, ch[:], op0=ALU.mult, op1=ALU.max)
                nc.vector.tensor_tensor(
                    ch[:], ch[:],
                    att_big_t[:, 0:TB * C].rearrange("p (t c) -> p t c", c=C),
                    op=ALU.mult)
                logit = ep.tile([128, TB * nh], f32, tag="logit")
                nc.vector.tensor_reduce(
                    logit[:].rearrange("p (t h) -> p t h", h=nh),
                    ch[:].rearrange("p t (h c) -> p t h c", h=nh),
                    axis=AX.X, op=ALU.add)
                pay = eb.tile([128, TB, CA], bf16, tag="ebuf")
                if fold:
                    elog_ap = pay[:, :, C:C + 1].rearrange(
                        "p t one -> p (t one)")
                else:
                    elog_t = ep.tile([128, TB * nh], bf16, tag="elog")
                    elog_ap = elog_t[:]
                nc.scalar.activation(elog_ap, logit[:], AF.Exp)
                nc.vector.tensor_tensor(
                    pay[:, :, 0:C].rearrange("p t (h c) -> p t h c", h=nh),
                    xl_u.rearrange("p t (h c) -> p t h c", h=nh),
                    elog_ap.rearrange("p (t h) -> p t h", h=nh)
                        .to_broadcast([128, TB, nh, C // nh]),
                    op=ALU.mult)

                accT = psum()
                if fold:
                    for t in range(TB):
                        nc.tensor.matmul(out=accT[0:CA, 0:128],
                                         lhsT=pay[:, t, :], rhs=Mt[:, t, :],
                                         start=(t == 0), stop=(t == TB - 1))
                else:
                    accD = psum()
                    for t in range(TB):
                        nc.tensor.matmul(out=accT[0:C, 0:128],
                                         lhsT=pay[:, t, :], rhs=Mt[:, t, :],
                                         start=(t == 0), stop=(t == TB - 1))
                        nc.tensor.matmul(out=accD[0:nh, 0:128],
                                         lhsT=elog_t[:, t * nh:(t + 1) * nh],
                                         rhs=Mt[:, t, :],
                                         start=(t == 0), stop=(t == TB - 1))

                # dense self-loop (channel-major)
                xlb = xlT_own[0:C, b * 128:(b + 1) * 128]
                xrb = xrT_own[0:C, b * 128:(b + 1) * 128]
                s_l = ep.tile([C, 128], bf16, tag="s_l")
                nc.vector.scalar_tensor_tensor(
                    s_l[:], xlb, wm_col[:], xrb, op0=ALU.add, op1=ALU.add)
                lr_l = ep.tile([C, 128], bf16, tag="lr_l")
                nc.vector.scalar_tensor_tensor(
                    lr_l[:], s_l[:], NEG, s_l[:], op0=ALU.mult, op1=ALU.max)
                lgl = psum()
                nc.tensor.matmul(out=lgl[0:CA if fold else nh, 0:128],
                                 lhsT=am_sb[0:C, 0:CA if fold else nh],
                                 rhs=lr_l[:], start=True, stop=True)
                if fold:
                    Q = ep.tile([CA, 128], f32, tag="Q")
                    nc.scalar.activation(Q[:], lgl[0:CA, 0:128], AF.Exp)
                    elbc = psum()
                    nc.tensor.matmul(out=elbc[0:C, 0:128],
                                     lhsT=hm_t[0:CA, 0:C], rhs=Q[:],
                                     start=True, stop=True)
                    nc.vector.tensor_tensor(Q[0:C, :], xlb,
                                            elbc[0:C, 0:128], op=ALU.mult)
                    nim = ep.tile([CA, 128], f32, tag="nim")
                    nc.vector.tensor_tensor(nim[:], accT[0:CA, 0:128], Q[:],
                                            op=ALU.add)
                    nc.vector.reciprocal(rec1[C:CA, :], nim[C:CA, :])
                    rbc = psum()
                    nc.tensor.matmul(out=rbc[0:C, 0:128],
                                     lhsT=hmsc_t[0:CA, 0:C], rhs=rec1[:],
                                     start=True, stop=True)
                    o1 = ep.tile([C, 128], f32, tag="o1")
                    nc.vector.tensor_tensor(o1[:], nim[0:C, :],
                                            rbc[0:C, 0:128], op=ALU.mult)
                else:
                    elog_l = ep.tile([nh, 128], f32, tag="elog_l")
                    nc.scalar.activation(elog_l[:], lgl[0:nh, 0:128], AF.Exp)
                    pay_l = ep.tile([C, 128], f32, tag="pay_l")
                    elbc = psum()
                    nc.tensor.matmul(out=elbc[0:C, 0:128],
                                     lhsT=hm_t[0:nh, 0:C],
                                     rhs=elog_l[:], start=True, stop=True)
                    nc.vector.tensor_tensor(pay_l[:], xlb, elbc[0:C, 0:128],
                                            op=ALU.mult)
                    nim = ep.tile([C, 128], f32, tag="nim")
                    nc.vector.tensor_tensor(nim[:], accT[0:C, 0:128],
                                            pay_l[:], op=ALU.add)
                    den = ep.tile([nh, 128], f32, tag="den")
                    nc.vector.tensor_tensor(den[:], accD[0:nh, 0:128],
                                            elog_l[:], op=ALU.add)
                    rec = ep.tile([nh, 128], f32, tag="rec")
                    nc.vector.reciprocal(rec[:], den[:])
                    rbc = psum()
                    nc.tensor.matmul(out=rbc[0:C, 0:128],
                                     lhsT=hmsc_t[0:nh, 0:C],
                                     rhs=rec[:], start=True, stop=True)
                    o1 = ep.tile([C, 128], f32, tag="o1")
                    nc.vector.tensor_tensor(o1[:], nim[:], rbc[0:C, 0:128],
                                            op=ALU.mult)
                nc.scalar.activation(h_out[0:C, b * 128:(b + 1) * 128],
                                     o1[:], AF.Relu, bias=pbb_col[:])
                if post_block is not None:
                    post_block(b)

        edge_layer(HC, HEADS, xl_tbl[0:HALF, :], xl_tbl[HALF:N, :],
                   We1_big, att1_big,
                   lambda b: xr1_own[:, b * 128:(b + 1) * 128],
                   xl1T, xr1T, wm1_col, cols["pbb1_col"], hmsc1,
                   am1_sb, hm1, h1T, False, None, l2_table_build)

        nc.gpsimd.collective_compute(
            "AllGather", ALU.bypass,
            replica_groups=[list(range(NC))],
            ins=[t2_stripe.ap().opt()],
            outs=[t2_full.ap().opt()])

        edge_layer(HID, 1, t2_full[0:HALF, :], t2_full[HALF:N, :],
                   We2_big, att2_big,
                   lambda b: st2_own[:, b * 2 * HID + HID:(b + 1) * 2 * HID],
                   xl2T, xr2T, wm2_col, cols["pbb2_col"], hmsc2,
                   am2_aug, hm2_aug, h2T, True, rec2_t, clf_block)

        for p in reversed(stack):
            p.__exit__(None, None, None)

    nc.compile()
    return nc


# ---------------------------------------------------------------- entry
def make_in_maps(inputs, maps):
    f = lambda k: np.asarray(inputs[k], np.float32)
    x = f("x")
    edge_attr = f("edge_attr")
    xT = np.ascontiguousarray(x.T.astype(ml_dtypes.bfloat16))
    ea_full = np.ascontiguousarray(edge_attr[:, 0].reshape(E // 128, 128).T)

    # host-computed columns
    bl1, br1, bias1 = f("c1_bl"), f("c1_br"), f("c1_bias")
    g1, b1, m1, v1 = f("bn1_gamma"), f("bn1_beta"), f("bn1_mean"), f("bn1_var")
    bl2, br2, bias2 = f("c2_bl"), f("c2_br"), f("c2_bias")
    g2, b2, m2, v2 = f("bn2_gamma"), f("bn2_beta"), f("bn2_mean"), f("bn2_var")
    col = lambda a: np.ascontiguousarray(a.reshape(-1, 1).astype(np.float32))

    hm1 = np.zeros((HEADS, HC), np.float32)
    for h in range(HEADS):
        hm1[h, h * HID:(h + 1) * HID] = 1.0
    am1 = np.zeros((HC, HEADS), np.float32)
    att1 = f("c1_att")
    for h in range(HEADS):
        am1[h * HID:(h + 1) * HID, h] = att1[h]
    am2 = np.zeros((HID, HID + 1), np.float32)
    am2[:, HID] = f("c2_att")[0]
    hm2 = np.zeros((HID + 1, HID), np.float32)
    hm2[HID, :] = 1.0

    common = dict(
        xT=xT, ea_full=ea_full,
        we1_col=col(f("c1_We")[0]), blbr1_col=col(bl1 + br1),
        pre1_col=col(bl1 + bias1 - m1),
        bnsc1_col=col(g1 / np.sqrt(v1 + EPS)),
        pbb1_col=col((bl1 + bias1 - m1) * (g1 / np.sqrt(v1 + EPS)) + b1),
        we2_col=col(f("c2_We")[0]), blbr2_col=col(bl2 + br2),
        pre2_col=col(bl2 + bias2 - m2),
        bnsc2_col=col(g2 / np.sqrt(v2 + EPS)),
        pbb2_col=col((bl2 + bias2 - m2) * (g2 / np.sqrt(v2 + EPS)) + b2),
        headmask1=np.ascontiguousarray(hm1),
        hmsc1=np.ascontiguousarray(hm1 * (g1 / np.sqrt(v1 + EPS))[None, :]),
        att_mask1=np.ascontiguousarray(am1.astype(ml_dtypes.bfloat16)),
        att_mask2=np.ascontiguousarray(am2.astype(ml_dtypes.bfloat16)),
        headmask2=np.ascontiguousarray(hm2),
        hmsc2=np.ascontiguousarray(hm2 * (g2 / np.sqrt(v2 + EPS))[None, :]),
        blbr1_row=np.ascontiguousarray((bl1 + br1).reshape(1, -1)),
        clf_b=np.ascontiguousarray(f("clf_b").reshape(-1, 1)),
    )
    for k in ["c1_Wl", "c1_Wr", "c2_Wl", "c2_Wr", "clf_W"]:
        common[k] = np.ascontiguousarray(f(k))
    for k in ["c1_We", "c1_att", "c2_We", "c2_att"]:
        common[k] = np.ascontiguousarray(f(k).reshape(1, -1))

    in_maps = []
    for c in range(NC):
        m = dict(maps[c])
        m.update(common)
        m["xTo"] = np.ascontiguousarray(xT[:, c * NPC:(c + 1) * NPC])
        in_maps.append(m)
    return in_maps


def kernel(**inputs):
    edge_index = np.asarray(inputs["edge_index"])
    edge_attr = np.asarray(inputs["edge_attr"], np.float32)

    maps, t_lo, t_hi = host_prep(edge_index, edge_attr)
    key = (t_lo, t_hi)
    if key not in _CACHE:
        _CACHE[key] = build(t_lo, t_hi)
    nc = _CACHE[key]

    in_maps = make_in_maps(inputs, maps)
    res = run_bass_kernel_spmd(nc, in_maps, core_ids=list(range(NC)))
    global LAST_RESULT
    LAST_RESULT = res
    out = np.concatenate(
        [np.ascontiguousarray(np.asarray(r["out_T"]).T)
         for r in res.results], axis=0)
    return out.astype(np.float32)


# revision 18
# speedup vs baseline: 1.1759x; 1.1069x over previous
"""GATv2 x2 + BN + classifier GNN on 8 trn2 NeuronCores.

Nodes are dst-sharded 6250/core; each core owns the edges pointing at its
nodes, grouped into 49 blocks of 128 dst nodes and padded to one uniform
tile count so all 8 cores run a single SPMD graph.  Per-edge xl/xr rows
are fetched with dma_gather spread round-robin over 4 SWDGE queues.  The
segment softmax + aggregation run through a per-tile one-hot Mt matrix on
the TensorEngine with the payload as the stationary operand, so the
aggregate lands CHANNEL-major ([C, dst]); softmax division, BN and ReLU
then use per-partition scalars, and the classifier is a plain matmul on
the channel-major hidden state.  Self-loops are applied densely at node
level.  Layer-2 tables are exchanged with an AllGather.
"""

import sys

sys.path.insert(0, "/opt/trn_rl_repo")

import numpy as np
import ml_dtypes

import bass_rust
import concourse.bass as bass
import concourse.bacc as bacc
import concourse.tile as tile
import concourse.mybir as mybir
from concourse.bass_utils import run_bass_kernel_spmd

f32 = mybir.dt.float32
bf16 = mybir.dt.bfloat16
i16 = mybir.dt.int16
AF = mybir.ActivationFunctionType
ALU = mybir.AluOpType
AX = mybir.AxisListType

N, E, IN, HID, HEADS, OUT = 50000, 800000, 128, 32, 4, 2
NEG = 0.2
EPS = 1e-5
NC = 8
NPC = N // NC                 # 6250
NBLK = (NPC + 127) // 128     # 49 (last block has 106 nodes)
HC = HEADS * HID              # 128
HALF = 25000                  # src-table split point (int16 idx range)

_CACHE = {}


# ---------------------------------------------------------------- host prep
def _wrap_idx(idx):
    """int [n] -> int16 [128, n//16]; token i at [i%16, i//16], replicated
    to all 8 Q7 core groups."""
    n = idx.shape[0]
    w = idx.astype(np.int16).reshape(n // 16, 16).T
    return np.ascontiguousarray(np.tile(w, (8, 1)))


def host_prep(edge_index, edge_attr):
    src = np.asarray(edge_index[0]).astype(np.int64)
    dst = np.asarray(edge_index[1]).astype(np.int64)
    ea = np.asarray(edge_attr[:, 0], np.float32)
    core_of = dst // NPC

    per_core = []
    t_lo = t_hi = 1
    for c in range(NC):
        m = core_of == c
        s_c, d_c, a_c = src[m], dst[m] - c * NPC, ea[m]
        blk = d_c // 128
        groups = []
        for b in range(NBLK):
            mb = blk == b
            mlo = mb & (s_c < HALF)
            mhi = mb & (s_c >= HALF)
            groups.append((s_c[mlo], d_c[mlo], a_c[mlo],
                           s_c[mhi], d_c[mhi], a_c[mhi]))
            t_lo = max(t_lo, (int(mlo.sum()) + 127) // 128)
            t_hi = max(t_hi, (int(mhi.sum()) + 127) // 128)
        per_core.append(groups)

    TB = t_lo + t_hi
    maps = []
    for c in range(NC):
        idx_lo = np.zeros((NBLK, t_lo * 128), np.int64)
        idx_hi = np.zeros((NBLK, t_hi * 128), np.int64)
        idx_dst = np.zeros((NBLK, TB * 128), np.int64)
        drel = np.full((NBLK, TB * 128), -1.0, np.float32)
        eatt = np.zeros((NBLK, TB * 128), np.float32)
        for b, (sl, dl, al, sh, dh, ah) in enumerate(per_core[c]):
            nl, nh = len(sl), len(sh)
            o = t_lo * 128
            idx_lo[b, :nl] = sl
            idx_hi[b, :nh] = sh - HALF
            idx_dst[b, :nl] = dl
            idx_dst[b, o:o + nh] = dh
            drel[b, :nl] = dl - b * 128
            drel[b, o:o + nh] = dh - b * 128
            eatt[b, :nl] = al
            eatt[b, o:o + nh] = ah
        d4 = drel.reshape(NBLK, TB, 128)          # [b, t, e]
        oneh = d4[:, :, :, None] == np.arange(128)[None, None, None, :]
        mt = oneh.transpose(2, 0, 1, 3)           # [e, b, t, d]
        mtt = oneh.transpose(3, 0, 1, 2)          # [d, b, t, e]
        maps.append(dict(
            idx_lo=_wrap_idx(idx_lo.reshape(-1)),
            idx_hi=_wrap_idx(idx_hi.reshape(-1)),
            idx_dst=_wrap_idx(idx_dst.reshape(-1)),
            mt_w=np.ascontiguousarray(
                mt.reshape(128, -1).astype(ml_dtypes.bfloat16)),
            mtt_w=np.ascontiguousarray(
                mtt.reshape(128, -1).astype(ml_dtypes.bfloat16)),
            eattr_w=np.ascontiguousarray(eatt.reshape(-1, 128).T),
        ))
    return maps, t_lo, t_hi


# ---------------------------------------------------------------- device
def build(t_lo, t_hi):
    TB = t_lo + t_hi
    TT = NBLK * TB
    nc = bacc.Bacc("TRN2", target_bir_lowering=False, debug=False,
                   num_devices=NC, num_swdge_queues=4)

    def din(name, shape, dt=f32):
        return nc.declare_dram_parameter(name, list(shape), dt, isOutput=False)

    xT = nc.declare_dram_parameter("xT", [128, N], bf16, isOutput=False)
    xTo = nc.declare_dram_parameter("xTo", [128, NPC], bf16, isOutput=False)
    ea_full = din("ea_full", [128, E // 128])
    idx_lo_d = din("idx_lo", [128, NBLK * t_lo * 8], i16)
    idx_hi_d = din("idx_hi", [128, NBLK * t_hi * 8], i16)
    idx_dst_d = din("idx_dst", [128, NBLK * TB * 8], i16)
    mt_d = nc.declare_dram_parameter("mt_w", [128, TT * 128], bf16,
                                     isOutput=False)
    mtt_d = nc.declare_dram_parameter("mtt_w", [128, TT * 128], bf16,
                                      isOutput=False)
    eattr_d = din("eattr_w", [128, TT])

    w = {k: din(k, sh) for k, sh in [
        ("c1_Wl", (IN, HC)), ("c1_Wr", (IN, HC)),
        ("c2_Wl", (HC, HID)), ("c2_Wr", (HC, HID)),
        ("clf_W", (HID, OUT)), ("clf_b", (OUT, 1)),
        ("c1_We", (1, HC)), ("c1_att", (1, HC)),
        ("c2_We", (1, HID)), ("c2_att", (1, HID)),
        # host-computed columns / masks
        ("we1_col", (HC, 1)), ("blbr1_col", (HC, 1)),
        ("pre1_col", (HC, 1)), ("bnsc1_col", (HC, 1)), ("pbb1_col", (HC, 1)),
        ("we2_col", (HID, 1)), ("blbr2_col", (HID, 1)),
        ("pre2_col", (HID, 1)), ("bnsc2_col", (HID, 1)), ("pbb2_col", (HID, 1)),
        ("headmask1", (HEADS, HC)), ("hmsc1", (HEADS, HC)),
        ("blbr1_row", (1, HC)),
    ]}
    am1 = nc.declare_dram_parameter("att_mask1", [HC, HEADS], bf16, isOutput=False)
    am2 = nc.declare_dram_parameter("att_mask2", [HID, HID + 1], bf16,
                                    isOutput=False)
    hm2_d = nc.declare_dram_parameter("headmask2", [HID + 1, HID], f32,
                                      isOutput=False)
    hmsc2_d = nc.declare_dram_parameter("hmsc2", [HID + 1, HID], f32,
                                        isOutput=False)
    out_T = nc.declare_dram_parameter("out_T", [OUT, NPC], f32, isOutput=True)

    xl_tbl = nc.dram_tensor("xl_tbl", [N, 128], bf16)
    xr_tbl = nc.dram_tensor("xr_tbl", [NPC, 128], bf16)
    t2_stripe = nc.dram_tensor("t2_stripe", [NPC, 128], bf16)
    t2_full = nc.dram_tensor("t2_full", [N, 128], bf16, addr_space="Shared")

    with tile.TileContext(nc) as tc:
        stack = []

        def pool(name, bufs, space="SBUF"):
            p = tc.tile_pool(name=name, bufs=bufs, space=space)
            stack.append(p)
            return p.__enter__()

        cst = pool("cst", 1)
        pp = pool("pp", 6, "PSUM")       # f32 psum (matmul accum)
        pb = pool("pb", 2, "PSUM")       # bf16 psum (transposes)

        def psum():
            return pp.tile([128, 512], f32, tag="ps", name="ps")

        def psum_bf():
            return pb.tile([128, 1024], bf16, tag="psb", name="psb")

        # ---- constants ----------------------------------------------------
        ones1f = cst.tile([1, 128], f32, tag="ones1f")
        nc.gpsimd.memset(ones1f[:], 1.0)

        rows = {}
        for k, wd in [("c1_We", HC), ("c1_att", HC),
                      ("c2_We", HID), ("c2_att", HID)]:
            t = cst.tile([1, wd], f32, tag=f"row_{k}")
            nc.sync.dma_start(t[:], w[k][:])
            rows[k] = t

        cols = {}
        for k, wd in [("we1_col", HC), ("blbr1_col", HC), ("pre1_col", HC),
                      ("bnsc1_col", HC), ("pbb1_col", HC),
                      ("we2_col", HID), ("blbr2_col", HID),
                      ("pre2_col", HID), ("bnsc2_col", HID),
                      ("pbb2_col", HID), ("clf_b", OUT)]:
            t = cst.tile([wd, 1], f32, tag=f"col_{k}")
            nc.sync.dma_start(t[:], (w[k] if k != "clf_b" else w["clf_b"])[:])
            cols[k] = t

        hm1 = cst.tile([HEADS, HC], f32, tag="hm1")
        nc.sync.dma_start(hm1[:], w["headmask1"][:])
        hmsc1 = cst.tile([HEADS, HC], f32, tag="hmsc1")
        nc.sync.dma_start(hmsc1[:], w["hmsc1"][:])
        am1_sb = cst.tile([HC, HEADS], bf16, tag="am1")
        nc.sync.dma_start(am1_sb[:], am1[:])
        am2_aug = cst.tile([HID, HID + 1], bf16, tag="am2")
        nc.sync.dma_start(am2_aug[:], am2[:])
        hm2_aug = cst.tile([HID + 1, HID], f32, tag="hm2")
        nc.sync.dma_start(hm2_aug[:], hm2_d[:])
        hmsc2 = cst.tile([HID + 1, HID], f32, tag="hmsc2")
        nc.sync.dma_start(hmsc2[:], hmsc2_d[:])
        rec2_t = cst.tile([HID + 1, 128], f32, tag="rec2")
        nc.gpsimd.memset(rec2_t[:], 1.0)
        blbr1_row = cst.tile([1, HC], f32, tag="blbr1_row")
        nc.sync.dma_start(blbr1_row[:], w["blbr1_row"][:])

        def rep(tag, row, wd, dt=bf16):
            ps = psum()
            nc.tensor.matmul(out=ps[:, :wd], lhsT=ones1f[:],
                             rhs=row[:, :wd], start=True, stop=True)
            t = cst.tile([128, wd], dt, tag=f"rep_{tag}")
            nc.vector.tensor_copy(t[:], ps[:, :wd])
            return t

        We1_rep = rep("We1", rows["c1_We"], HC)
        We2_rep = rep("We2", rows["c2_We"], HID)
        att1_rep = rep("att1", rows["c1_att"], HC)
        att2_rep = rep("att2", rows["c2_att"], HID)
        blbr1_rep = rep("blbr1", blbr1_row, HC, f32)

        def big(tag, base, wd):
            t = cst.tile([128, TB * wd], bf16, tag=f"big_{tag}")
            for i in range(TB):
                nc.sync.dma_start(t[:, i * wd:(i + 1) * wd], base[:])
            return t

        att1_big = big("att1", att1_rep, HC)
        att2_big = big("att2", att2_rep, HID)
        We1_big = big("We1", We1_rep, HC)
        We2_big = big("We2", We2_rep, HID)

        iota_i = cst.tile([128, 128], i16, tag="iota_i")
        nc.gpsimd.iota(iota_i[:], pattern=[[1, 128]], base=0,
                       channel_multiplier=0)
        iota_rep = cst.tile([128, 128], bf16, tag="iota_rep")
        nc.vector.tensor_copy(iota_rep[:], iota_i[:])
        icol_i = cst.tile([128, 1], i16, tag="icol_i")
        nc.gpsimd.iota(icol_i[:], pattern=[[1, 1]], base=0,
                       channel_multiplier=1)
        icol_f = cst.tile([128, 1], f32, tag="icol_f")
        nc.vector.tensor_copy(icol_f[:], icol_i[:])
        ident = cst.tile([128, 128], bf16, tag="ident")
        nc.vector.tensor_scalar(ident[:], iota_rep[:], icol_f[:], None,
                                op0=ALU.is_equal)

        # mean of edge_attr
        with tc.tile_pool(name="eaf", bufs=1) as eaf:
            ea_sb = eaf.tile([128, E // 128], f32, tag="ea")
            nc.sync.dma_start(ea_sb[:], ea_full[:])
            ea_part = cst.tile([128, 1], f32, tag="ea_part")
            nc.vector.tensor_reduce(ea_part[:], ea_sb[:], axis=AX.X,
                                    op=ALU.add)
        ea_sum = cst.tile([128, 1], f32, tag="ea_sum")
        nc.gpsimd.partition_all_reduce(ea_sum[:], ea_part[:], 128,
                                       bass_rust.ReduceOp.add)
        mean_sc = cst.tile([128, 1], f32, tag="mean_sc")
        nc.vector.tensor_scalar(mean_sc[:], ea_sum[:], 1.0 / E, None,
                                op0=ALU.mult)
        wm1_col = cst.tile([HC, 1], f32, tag="wm1_col")
        nc.vector.tensor_tensor(wm1_col[:], cols["we1_col"][:], mean_sc[0:HC],
                                op=ALU.mult)
        wm2_col = cst.tile([HID, 1], f32, tag="wm2_col")
        nc.vector.tensor_tensor(wm2_col[:], cols["we2_col"][:], mean_sc[0:HID],
                                op=ALU.mult)

        wcat1 = cst.tile([128, 2 * HC], bf16, tag="wcat1")
        nc.gpsimd.dma_start(wcat1[:, 0:HC], w["c1_Wl"][:])
        nc.gpsimd.dma_start(wcat1[:, HC:2 * HC], w["c1_Wr"][:])
        wcat2 = cst.tile([128, 2 * HID], bf16, tag="wcat2")
        nc.gpsimd.dma_start(wcat2[:, 0:HID], w["c2_Wl"][:])
        nc.gpsimd.dma_start(wcat2[:, HID:2 * HID], w["c2_Wr"][:])
        clfW = cst.tile([HID, OUT], bf16, tag="clfW")
        nc.gpsimd.dma_start(clfW[:], w["clf_W"][:])

        meta = pool("meta", 1)
        ilo_sb = meta.tile([128, NBLK * t_lo * 8], i16, tag="ilo")
        nc.sync.dma_start(ilo_sb[:], idx_lo_d[:])
        ihi_sb = meta.tile([128, NBLK * t_hi * 8], i16, tag="ihi")
        nc.sync.dma_start(ihi_sb[:], idx_hi_d[:])
        idst_sb = meta.tile([128, NBLK * TB * 8], i16, tag="idst")
        nc.sync.dma_start(idst_sb[:], idx_dst_d[:])
        eattr_bf = meta.tile([128, TT], bf16, tag="eattr_bf")
        with tc.tile_pool(name="mf32", bufs=1) as mf32:
            eattr_sb = mf32.tile([128, TT], f32, tag="eattr")
            nc.sync.dma_start(eattr_sb[:], eattr_d[:])
            nc.vector.tensor_copy(eattr_bf[:], eattr_sb[:])

        # ---- phase A: xl table (node-major rows for gather) ---------------
        CH = 1024
        keep = pool("keep", 1)
        # channel-major own-node transforms (for self-loops / L2 tables)
        xl1T = keep.tile([128, NBLK * 128], bf16, tag="xl1T")
        xr1T = keep.tile([128, NBLK * 128], bf16, tag="xr1T")
        h1T = keep.tile([128, NBLK * 128], bf16, tag="h1T")
        for t_ in (xl1T, xr1T, h1T):
            nc.gpsimd.memset(t_[:], 0.0)
        # L2 aliases (L1 contents are dead by the time these are written)
        xl2T = xl1T
        xr2T = xr1T
        h2T = h1T
        st2_own = keep.tile([128, NBLK * 2 * HID], bf16, tag="st2_own")

        xa = pool("xa", 2)
        for ci in range((N + CH - 1) // CH):
            n0 = ci * CH
            n1 = min(n0 + CH, N)
            nfull = (n1 - n0) // 128
            xTs = xa.tile([128, CH], bf16, tag="xTs")
            nc.sync.dma_start(xTs[:, :n1 - n0], xT[:, n0:n1])
            stg = xa.tile([128, CH // 128, 128], bf16, tag="stage")
            for t in range((n1 - n0 + 127) // 128):
                m = min(128, n1 - n0 - t * 128)
                ps = psum()
                nc.tensor.matmul(out=ps[:m, 0:HC],
                                 lhsT=xTs[:, t * 128:t * 128 + m],
                                 rhs=wcat1[:, 0:HC], start=True, stop=True)
                if t % 2 == 0:
                    nc.scalar.activation(stg[:m, t, :], ps[:m, 0:HC],
                                         AF.Identity)
                else:
                    nc.vector.tensor_copy(stg[:m, t, :], ps[:m, 0:HC])
            if nfull:
                nc.sync.dma_start(
                    xl_tbl[n0:n0 + nfull * 128, :].rearrange(
                        "(t p) c -> p t c", p=128),
                    stg[:, 0:nfull, :])
            if (n1 - n0) % 128:
                m = (n1 - n0) % 128
                nc.sync.dma_start(xl_tbl[n0 + nfull * 128:n1, :],
                                  stg[:m, nfull, :])

        ob = pool("ob", 2)
        xTos = ob.tile([128, NPC], bf16, tag="xTos", bufs=1)
        nc.sync.dma_start(xTos[:], xTo[:])
        for b in range(NBLK):
            m = min(128, NPC - b * 128)
            # node-major xr rows (for the L1 xr gather)
            ps = psum()
            nc.tensor.matmul(out=ps[:m, 0:HC],
                             lhsT=xTos[:, b * 128:b * 128 + m],
                             rhs=wcat1[:, HC:2 * HC], start=True, stop=True)
            xrst = ob.tile([128, 128], bf16, tag="xrst")
            nc.vector.tensor_tensor(xrst[:m, :], ps[:m, 0:HC],
                                    blbr1_rep[:m, :], op=ALU.add)
            nc.sync.dma_start(xr_tbl[b * 128:b * 128 + m, :], xrst[:m, :])
            # channel-major own transforms
            ps2 = psum()
            nc.tensor.matmul(out=ps2[:, 0:m], lhsT=wcat1[:, 0:HC],
                             rhs=xTos[:, b * 128:b * 128 + m],
                             start=True, stop=True)
            nc.vector.tensor_copy(xl1T[:, b * 128:b * 128 + m],
                                  ps2[:, 0:m])
            ps3 = psum()
            nc.tensor.matmul(out=ps3[:, 0:m], lhsT=wcat1[:, HC:2 * HC],
                             rhs=xTos[:, b * 128:b * 128 + m],
                             start=True, stop=True)
            nc.vector.tensor_scalar(xr1T[:, b * 128:b * 128 + m],
                                    ps3[:, 0:m], cols["blbr1_col"][:], None,
                                    op0=ALU.add)

        # ---- edge phase ----------------------------------------------------
        ep = pool("ep", 3)
        eb = pool("eb", 4)
        mq = pool("mq", 3)
        l2 = pool("l2", 2)
        fin = pool("fin", 2)
        QN = [0]

        def l2_table_build(b):
            ps = psum()
            nc.tensor.matmul(out=ps[0:HID, 0:128], lhsT=wcat2[:, 0:HID],
                             rhs=h1T[:, b * 128:(b + 1) * 128],
                             start=True, stop=True)
            nc.vector.tensor_copy(xl2T[0:HID, b * 128:(b + 1) * 128],
                                  ps[0:HID, 0:128])
            ps2 = psum()
            nc.tensor.matmul(out=ps2[0:HID, 0:128], lhsT=wcat2[:, HID:2 * HID],
                             rhs=h1T[:, b * 128:(b + 1) * 128],
                             start=True, stop=True)
            nc.vector.tensor_scalar(xr2T[0:HID, b * 128:(b + 1) * 128],
                                    ps2[0:HID, 0:128],
                                    cols["blbr2_col"][:], None, op0=ALU.add)
            tps = psum_bf()
            nc.tensor.transpose(tps[:, 0:HID],
                                xl2T[0:HID, b * 128:(b + 1) * 128],
                                ident[0:HID, 0:HID])
            nc.tensor.transpose(tps[:, HID:2 * HID],
                                xr2T[0:HID, b * 128:(b + 1) * 128],
                                ident[0:HID, 0:HID])
            nc.vector.tensor_copy(st2_own[:, b * 2 * HID:(b + 1) * 2 * HID],
                                  tps[:, 0:2 * HID])
            m = min(128, NPC - b * 128)
            nc.sync.dma_start(
                t2_stripe[b * 128:b * 128 + m, 0:2 * HID],
                st2_own[0:m, b * 2 * HID:(b + 1) * 2 * HID])

        def clf_block(b):
            m = min(128, NPC - b * 128)
            ps = psum()
            nc.tensor.matmul(out=ps[0:OUT, 0:m], lhsT=clfW[:],
                             rhs=h2T[0:HID, b * 128:b * 128 + m],
                             start=True, stop=True)
            ot = fin.tile([OUT, 128], f32, tag="ot")
            nc.scalar.activation(ot[:, 0:m], ps[0:OUT, 0:m], AF.Identity,
                                 bias=cols["clf_b"][:])
            nc.sync.dma_start(out_T[:, b * 128:b * 128 + m], ot[:, 0:m])

        def edge_layer(C, nh, tbl_lo, tbl_hi, We_big_t, att_big_t, xr_rhs,
                       tbl_dst, xr_cols,
                       xlT_own, xrT_own, wm_col, pbb_col, hmsc_t,
                       am_sb, hm_t, h_out, fold, rec1, post_block):
            CA = C + 1 if fold else C
            for b in range(NBLK):
                xlg = ep.tile([128, TB, 128], bf16, tag="xlg")

                def gchunk(dst_t, toff, ntile, tbl, isrc, coff):
                    done = 0
                    while done < ntile:
                        k = min(8, ntile - done)
                        nc.gpsimd.dma_gather(
                            dst_t[:, toff + done:toff + done + k, :], tbl,
                            isrc[:, coff + done * 8:coff + (done + k) * 8],
                            k * 128, k * 128, 128, elem_step=128,
                            queue_num=QN[0] % 4)
                        QN[0] += 1
                        done += k

                gchunk(xlg, 0, t_lo, tbl_lo, ilo_sb, b * t_lo * 8)
                gchunk(xlg, t_lo, t_hi, tbl_hi, ihi_sb, b * t_hi * 8)
                Mt = mq.tile([128, TB, 128], bf16, tag="mt")
                nc.sync.dma_start(
                    Mt[:], mt_d[:, b * TB * 128:(b + 1) * TB * 128]
                    .rearrange("p (t d) -> p t d", d=128))
                xl_u = xlg[:, :, 0:C]

                if xr_rhs is None:
                    # xr[dst] per edge gathered from node-major rows
                    xrg = ep.tile([128, TB, 128], bf16, tag="xre")
                    gchunk(xrg, 0, TB, tbl_dst, idst_sb, b * TB * 8)
                    xr_e = xrg[:, :, xr_cols[0]:xr_cols[1]]
                else:
                    MtT = mq.tile([128, TB, 128], bf16, tag="mtt")
                    nc.sync.dma_start(
                        MtT[:], mtt_d[:, b * TB * 128:(b + 1) * TB * 128]
                        .rearrange("p (t e) -> p t e", e=128))
                    xre_t = ep.tile([128, TB, C], bf16, tag="xre")
                    for t in range(TB):
                        xps = psum()
                        nc.tensor.matmul(out=xps[:, 0:C], lhsT=MtT[:, t, :],
                                         rhs=xr_rhs(b), start=True, stop=True)
                        if t % 2 == 0:
                            nc.scalar.activation(xre_t[:, t, :], xps[:, 0:C],
                                                 AF.Identity)
                        else:
                            nc.vector.tensor_copy(xre_t[:, t, :],
                                                  xps[:, 0:C])
                    xr_e = xre_t[:, :, 0:C]

                ea_bc = eattr_bf[:, b * TB:(b + 1) * TB].to_broadcast(
                    [128, TB, C])
                ch = eb.tile([128, TB, C], bf16, tag="ebuf")
                nc.vector.tensor_tensor(
                    ch[:], We_big_t[:, 0:TB * C].rearrange(
                        "p (t c) -> p t c", c=C), ea_bc, op=ALU.mult)
                nc.vector.tensor_tensor(ch[:], ch[:], xl_u, op=ALU.add)
                nc.vector.tensor_tensor(ch[:], ch[:], xr_e, op=ALU.add)
                nc.vector.scalar_tensor_tensor(
                    ch[:], ch[:], NEG, ch[:], op0=ALU.mult, op1=ALU.max)
                nc.vector.tensor_tensor(
                    ch[:], ch[:],
                    att_big_t[:, 0:TB * C].rearrange("p (t c) -> p t c", c=C),
                    op=ALU.mult)
                logit = ep.tile([128, TB * nh], f32, tag="logit")
                nc.vector.tensor_reduce(
                    logit[:].rearrange("p (t h) -> p t h", h=nh),
                    ch[:].rearrange("p t (h c) -> p t h c", h=nh),
                    axis=AX.X, op=ALU.add)
                pay = eb.tile([128, TB, CA], bf16, tag="ebuf")
                if fold:
                    elog_ap = pay[:, :, C:C + 1].rearrange(
                        "p t one -> p (t one)")
                else:
                    elog_t = ep.tile([128, TB * nh], bf16, tag="elog")
                    elog_ap = elog_t[:]
                nc.scalar.activation(elog_ap, logit[:], AF.Exp)
                nc.vector.tensor_tensor(
                    pay[:, :, 0:C].rearrange("p t (h c) -> p t h c", h=nh),
                    xl_u.rearrange("p t (h c) -> p t h c", h=nh),
                    elog_ap.rearrange("p (t h) -> p t h", h=nh)
                        .to_broadcast([128, TB, nh, C // nh]),
                    op=ALU.mult)

                accT = psum()
                if fold:
                    for t in range(TB):
                        nc.tensor.matmul(out=accT[0:CA, 0:128],
                                         lhsT=pay[:, t, :], rhs=Mt[:, t, :],
                                         start=(t == 0), stop=(t == TB - 1))
                else:
                    accD = psum()
                    for t in range(TB):
                        nc.tensor.matmul(out=accT[0:C, 0:128],
                                         lhsT=pay[:, t, :], rhs=Mt[:, t, :],
                                         start=(t == 0), stop=(t == TB - 1))
                        nc.tensor.matmul(out=accD[0:nh, 0:128],
                                         lhsT=elog_t[:, t * nh:(t + 1) * nh],
                                         rhs=Mt[:, t, :],
                                         start=(t == 0), stop=(t == TB - 1))

                # dense self-loop (channel-major)
                xlb = xlT_own[0:C, b * 128:(b + 1) * 128]
                xrb = xrT_own[0:C, b * 128:(b + 1) * 128]
                s_l = ep.tile([C, 128], bf16, tag="s_l")
                nc.vector.scalar_tensor_tensor(
                    s_l[:], xlb, wm_col[:], xrb, op0=ALU.add, op1=ALU.add)
                lr_l = ep.tile([C, 128], bf16, tag="lr_l")
                nc.vector.scalar_tensor_tensor(
                    lr_l[:], s_l[:], NEG, s_l[:], op0=ALU.mult, op1=ALU.max)
                lgl = psum()
                nc.tensor.matmul(out=lgl[0:CA if fold else nh, 0:128],
                                 lhsT=am_sb[0:C, 0:CA if fold else nh],
                                 rhs=lr_l[:], start=True, stop=True)
                if fold:
                    Q = ep.tile([CA, 128], f32, tag="Q")
                    nc.scalar.activation(Q[:], lgl[0:CA, 0:128], AF.Exp)
                    elbc = psum()
                    nc.tensor.matmul(out=elbc[0:C, 0:128],
                                     lhsT=hm_t[0:CA, 0:C], rhs=Q[:],
                                     start=True, stop=True)
                    nc.vector.tensor_tensor(Q[0:C, :], xlb,
                                            elbc[0:C, 0:128], op=ALU.mult)
                    nim = ep.tile([CA, 128], f32, tag="nim")
                    nc.vector.tensor_tensor(nim[:], accT[0:CA, 0:128], Q[:],
                                            op=ALU.add)
                    nc.vector.reciprocal(rec1[C:CA, :], nim[C:CA, :])
                    rbc = psum()
                    nc.tensor.matmul(out=rbc[0:C, 0:128],
                                     lhsT=hmsc_t[0:CA, 0:C], rhs=rec1[:],
                                     start=True, stop=True)
                    o1 = ep.tile([C, 128], f32, tag="o1")
                    nc.vector.tensor_tensor(o1[:], nim[0:C, :],
                                            rbc[0:C, 0:128], op=ALU.mult)
                else:
                    elog_l = ep.tile([nh, 128], f32, tag="elog_l")
                    nc.scalar.activation(elog_l[:], lgl[0:nh, 0:128], AF.Exp)
                    pay_l = ep.tile([C, 128], f32, tag="pay_l")
                    elbc = psum()
                    nc.tensor.matmul(out=elbc[0:C, 0:128],
                                     lhsT=hm_t[0:nh, 0:C],
                                     rhs=elog_l[:], start=True, stop=True)
                    nc.vector.tensor_tensor(pay_l[:], xlb, elbc[0:C, 0:128],
                                            op=ALU.mult)
                    nim = ep.tile([C, 128], f32, tag="nim")
                    nc.vector.tensor_tensor(nim[:], accT[0:C, 0:128],
                                            pay_l[:], op=ALU.add)
                    den = ep.tile([nh, 128], f32, tag="den")
                    nc.vector.tensor_tensor(den[:], accD[0:nh, 0:128],
                                            elog_l[:], op=ALU.add)
                    rec = ep.tile([nh, 128], f32, tag="rec")
                    nc.vector.reciprocal(rec[:], den[:])
                    rbc = psum()
                    nc.tensor.matmul(out=rbc[0:C, 0:128],
                                     lhsT=hmsc_t[0:nh, 0:C],
                                     rhs=rec[:], start=True, stop=True)
                    o1 = ep.tile([C, 128], f32, tag="o1")
                    nc.vector.tensor_tensor(o1[:], nim[:], rbc[0:C, 0:128],
                                            op=ALU.mult)
                nc.scalar.activation(h_out[0:C, b * 128:(b + 1) * 128],
                                     o1[:], AF.Relu, bias=pbb_col[:])
                if post_block is not None:
                    post_block(b)

        edge_layer(HC, HEADS, xl_tbl[0:HALF, :], xl_tbl[HALF:N, :],
                   We1_big, att1_big, None, xr_tbl[:, :], (0, 128),
                   xl1T, xr1T, wm1_col, cols["pbb1_col"], hmsc1,
                   am1_sb, hm1, h1T, False, None, l2_table_build)

        nc.gpsimd.collective_compute(
            "AllGather", ALU.bypass,
            replica_groups=[list(range(NC))],
            ins=[t2_stripe.ap().opt()],
            outs=[t2_full.ap().opt()])

        edge_layer(HID, 1, t2_full[0:HALF, :], t2_full[HALF:N, :],
                   We2_big, att2_big,
                   lambda b: st2_own[:, b * 2 * HID + HID:(b + 1) * 2 * HID],
                   None, None,
                   xl2T, xr2T, wm2_col, cols["pbb2_col"], hmsc2,
                   am2_aug, hm2_aug, h2T, True, rec2_t, clf_block)

        for p in reversed(stack):
            p.__exit__(None, None, None)

    nc.compile()
    return nc


# ---------------------------------------------------------------- entry
def make_in_maps(inputs, maps):
    f = lambda k: np.asarray(inputs[k], np.float32)
    x = f("x")
    edge_attr = f("edge_attr")
    xT = np.ascontiguousarray(x.T.astype(ml_dtypes.bfloat16))
    ea_full = np.ascontiguousarray(edge_attr[:, 0].reshape(E // 128, 128).T)

    # host-computed columns
    bl1, br1, bias1 = f("c1_bl"), f("c1_br"), f("c1_bias")
    g1, b1, m1, v1 = f("bn1_gamma"), f("bn1_beta"), f("bn1_mean"), f("bn1_var")
    bl2, br2, bias2 = f("c2_bl"), f("c2_br"), f("c2_bias")
    g2, b2, m2, v2 = f("bn2_gamma"), f("bn2_beta"), f("bn2_mean"), f("bn2_var")
    col = lambda a: np.ascontiguousarray(a.reshape(-1, 1).astype(np.float32))

    hm1 = np.zeros((HEADS, HC), np.float32)
    for h in range(HEADS):
        hm1[h, h * HID:(h + 1) * HID] = 1.0
    am1 = np.zeros((HC, HEADS), np.float32)
    att1 = f("c1_att")
    for h in range(HEADS):
        am1[h * HID:(h + 1) * HID, h] = att1[h]
    am2 = np.zeros((HID, HID + 1), np.float32)
    am2[:, HID] = f("c2_att")[0]
    hm2 = np.zeros((HID + 1, HID), np.float32)
    hm2[HID, :] = 1.0

    common = dict(
        xT=xT, ea_full=ea_full,
        we1_col=col(f("c1_We")[0]), blbr1_col=col(bl1 + br1),
        pre1_col=col(bl1 + bias1 - m1),
        bnsc1_col=col(g1 / np.sqrt(v1 + EPS)),
        pbb1_col=col((bl1 + bias1 - m1) * (g1 / np.sqrt(v1 + EPS)) + b1),
        we2_col=col(f("c2_We")[0]), blbr2_col=col(bl2 + br2),
        pre2_col=col(bl2 + bias2 - m2),
        bnsc2_col=col(g2 / np.sqrt(v2 + EPS)),
        pbb2_col=col((bl2 + bias2 - m2) * (g2 / np.sqrt(v2 + EPS)) + b2),
        headmask1=np.ascontiguousarray(hm1),
        hmsc1=np.ascontiguousarray(hm1 * (g1 / np.sqrt(v1 + EPS))[None, :]),
        att_mask1=np.ascontiguousarray(am1.astype(ml_dtypes.bfloat16)),
        att_mask2=np.ascontiguousarray(am2.astype(ml_dtypes.bfloat16)),
        headmask2=np.ascontiguousarray(hm2),
        hmsc2=np.ascontiguousarray(hm2 * (g2 / np.sqrt(v2 + EPS))[None, :]),
        blbr1_row=np.ascontiguousarray((bl1 + br1).reshape(1, -1)),
        clf_b=np.ascontiguousarray(f("clf_b").reshape(-1, 1)),
    )
    for k in ["c1_Wl", "c1_Wr", "c2_Wl", "c2_Wr", "clf_W"]:
        common[k] = np.ascontiguousarray(f(k))
    for k in ["c1_We", "c1_att", "c2_We", "c2_att"]:
        common[k] = np.ascontiguousarray(f(k).reshape(1, -1))

    in_maps = []
    for c in range(NC):
        m = dict(maps[c])
        m.update(common)
        m["xTo"] = np.ascontiguousarray(xT[:, c * NPC:(c + 1) * NPC])
        in_maps.append(m)
    return in_maps


def kernel(**inputs):
    edge_index = np.asarray(inputs["edge_index"])
    edge_attr = np.asarray(inputs["edge_attr"], np.float32)

    maps, t_lo, t_hi = host_prep(edge_index, edge_attr)
    key = (t_lo, t_hi)
    if key not in _CACHE:
        _CACHE[key] = build(t_lo, t_hi)
    nc = _CACHE[key]

    in_maps = make_in_maps(inputs, maps)
    res = run_bass_kernel_spmd(nc, in_maps, core_ids=list(range(NC)))
    global LAST_RESULT
    LAST_RESULT = res
    out = np.concatenate(
        [np.ascontiguousarray(np.asarray(r["out_T"]).T)
         for r in res.results], axis=0)
    return out.astype(np.float32)
